# revision 12
# baseline (speedup 1.0000x reference)
"""Trainium2 Bass kernel for nn_EnhancedCardAwarePolicy.

Strategy: pure data-parallel across 8 NeuronCores (batch 16384 -> 2048/core).

Key algebraic simplifications (exactly value-preserving vs the reference):
  * The hand self-attention MHA is dead code: the cross-attention that
    consumes it has sequence length 1, so its softmax is identically 1 and
    its output is independent of the query.  hand_ctx reduces to
        (8 / max(hand_size,1)) * (enemy_emb @ he_wv @ he_wo + he_bv @ he_wo + he_bo)
  * Card encodings are pure functions of the card index 0..53 -> fold the
    embedding tables into one [54, 32] table, and fold that table through
    the downstream linear layers, so the enemy-card path becomes two
    matmuls against a one-hot [54, B] matrix built on-device.
  * strat_ctx's second linear layer is folded into cx_w1.
  * The per-action tables ([30,4] action_card_indices, replicated per the
    sharding hint) are folded into per-action bias vectors on the host.
  * softmax+bonus is computed unnormalized: out = score + (expl@Bm)/(expl@1),
    with the scalar bias as_b3 folded so one reciprocal+mul suffices.

Device layout: feature-major [D, B] activations; per-batch card features are
computed batch-major [128, 16] on GPSIMD and rotated into rows with PE
transposes.  The dominant cost (the [B, 30, 64/32] action MLP) runs as
2-actions-per-128-partition matmuls with relu+bias fused into the PSUM->SBUF
eviction, split across the Scalar and Vector engines.
"""

import numpy as np
from contextlib import ExitStack

B = 16384
NCORES = 8
BC = B // NCORES          # 2048 batch rows per core
NCH = 4                   # chunks per core
N = BC // NCH             # 512 batch columns per chunk
A = 30                    # real actions
AP_ = 32                  # padded actions
E = 32
HID = 128

_cache = {}


# ---------------------------------------------------------------------------
# host-side folding
# ---------------------------------------------------------------------------

def _card_table(val_emb, suit_emb, type_emb):
    """[54, 32] full card encoding table, matching _encode_cards."""
    c = np.arange(54)
    invalid = (c == 0) | (c == 53)
    v = np.where(invalid, 0, (c - 1) % 13 + 1)
    s = np.where(invalid, 0, (c - 1) // 13 + 1)
    ce = np.concatenate([val_emb[v], suit_emb[s]], axis=-1)          # [54, 32]
    ct = np.where(v == 11, 1, np.where(v == 12, 2, np.where(v == 13, 3, 0)))
    te = type_emb[ct]                                                # [54, 8]
    pad = np.zeros((54, E - te.shape[-1]), np.float32)
    return (ce + np.concatenate([te, pad], axis=-1)).astype(np.float32)


def _action_fold(ac, card_emb, ce_w1, ce_b1, ce_w2, ce_b2,
                 as_w1, as_b1, as_b3):
    """Per-action biases + bonus matrix from action_card_indices [30, 4]."""
    ac = np.asarray(ac, np.int64)
    mask = ac != 0
    combo_size = mask.sum(1).astype(np.float32)
    values = np.where(mask, (ac - 1) % 13 + 1, 0)
    has_valid = mask.any(1)
    fidx = np.argmax(mask, axis=1)
    fv = values[np.arange(ac.shape[0]), fidx]
    same = np.where(mask, values == fv[:, None], True).all(1).astype(np.float32)
    vf = values.astype(np.float32)
    attack = np.where(values == 1, 1.0,
             np.where(values == 11, 10.0,
             np.where(values == 12, 15.0,
             np.where(values == 13, 20.0, vf))))
    total = (attack * mask).sum(1).astype(np.float32)
    suits = np.where(mask, (ac - 1) // 13 + 1, 0)
    uniq = sum((suits == s).any(1) for s in (1, 2, 3, 4)).astype(np.float32)
    ace = ((values == 1) & mask).any(1).astype(np.float32)
    valid = ((combo_size <= 4.0) & ((same > 0) | (ace > 0))).astype(np.float32)
    feats = np.stack([combo_size, same, total, uniq, ace, valid], 1)
    feats = np.where(has_valid[:, None], feats, 0.0).astype(np.float32)

    emb = card_emb[ac]                                   # [30, 4, 32]
    m = mask.astype(np.float32)[..., None]
    cnt = np.maximum(m.sum(1), 1.0)
    act_emb = (emb * m).sum(1) / cnt
    act_emb = np.where(has_valid[:, None], act_emb, 0.0).astype(np.float32)
    combo_enc = np.maximum(feats @ ce_w1 + ce_b1, 0.0) @ ce_w2 + ce_b2

    action_bias = act_emb @ as_w1[HID:HID + E] + combo_enc @ as_w1[HID + E:] + as_b1

    strength = feats[:, 2] / 20.0
    b3 = float(as_b3[0])
    Bm1 = np.zeros((4, AP_ + 1), np.float32)
    for a in range(A):
        if has_valid[a]:
            col = np.array([strength[a], 1.0 - strength[a], 0.0, 0.0])
        else:
            col = np.array([0.0, 0.0, 0.0, 2.0])
        Bm1[:, a] = col + b3
    Bm1[:, AP_] = 1.0                                    # denominator column
    ab = np.zeros((AP_, 64), np.float32)
    ab[:A] = action_bias
    return ab, Bm1


def _prep(inputs):
    """Fold weights, build per-core input maps. Returns (in_maps, shapes)."""
    f32 = lambda x: np.ascontiguousarray(np.asarray(x), dtype=np.float32)
    hc = np.asarray(inputs["hand_cards"])        # [B, 8] int
    ec = np.asarray(inputs["enemy_card"])        # [B]
    hs = np.asarray(inputs["hand_size"])         # [B]
    gs = f32(inputs["game_state"])               # [B, 10]
    dc = f32(inputs["discard_pile_cards"])       # [B, 54]

    card_emb = _card_table(f32(inputs["val_emb"]), f32(inputs["suit_emb"]),
                           f32(inputs["type_emb"]))
    card_emb1 = np.concatenate([card_emb, np.ones((54, 1), np.float32)], 1)

    he_wv, he_bv = f32(inputs["he_wv"]), f32(inputs["he_bv"])
    he_wo, he_bo = f32(inputs["he_wo"]), f32(inputs["he_bo"])
    Mc = np.concatenate([he_wv @ he_wo, (he_bv @ he_wo + he_bo)[None]], 0)  # [33,32]
    A0s = 8.0 * (card_emb1 @ Mc)                                   # [54, 32]

    cx_w1, cx_b1 = f32(inputs["cx_w1"]), f32(inputs["cx_b1"])
    W1h = np.ascontiguousarray(cx_w1[0:E])                         # [32, 128]
    A2 = card_emb @ cx_w1[E:2 * E]                                 # [54, 128]
    W1s = cx_w1[2 * E:2 * E + 32]                                  # [32, 128]
    W1d = np.ascontiguousarray(cx_w1[2 * E + 32:])                 # [54, 128]
    se_w1, se_b1 = f32(inputs["se_w1"]).copy(), f32(inputs["se_b1"])
    se_w2, se_b2 = f32(inputs["se_w2"]), f32(inputs["se_b2"])
    U = se_w2 @ W1s                                                # [64, 128]
    b1f = cx_b1 + se_b2 @ W1s                                      # [128]
    se_w1[19] /= 4.0          # device computes suit-diversity count 0..4

    as_w1, as_b1 = f32(inputs["as_w1"]), f32(inputs["as_b1"])
    as_w2, as_b2 = f32(inputs["as_w2"]), f32(inputs["as_b2"])
    as_w3, as_b3 = f32(inputs["as_w3"]), f32(inputs["as_b3"])
    ab, Bm1 = _action_fold(inputs["action_card_indices"], card_emb,
                           f32(inputs["ce_w1"]), f32(inputs["ce_b1"]),
                           f32(inputs["ce_w2"]), f32(inputs["ce_b2"]),
                           as_w1, as_b1, as_b3)
    W1c = as_w1[:HID]                                              # [128, 64]
    W1cd = np.concatenate([W1c, W1c], 1)                           # [128, 128]
    abp = np.zeros((128, 16), np.float32)
    for p in range(16):
        abp[0:64, p] = ab[2 * p]
        abp[64:128, p] = ab[2 * p + 1]
    W2blk = np.zeros((128, 64), np.float32)
    W2blk[0:64, 0:32] = as_w2
    W2blk[64:128, 32:64] = as_w2
    b2q = np.tile(as_b2, 4)[:, None].astype(np.float32)            # [128, 1]
    w3blk = np.zeros((128, 4), np.float32)
    for i in range(4):
        w3blk[32 * i:32 * i + 32, i] = as_w3[:, 0]

    consts = {
        "iota54": np.arange(54, dtype=np.float32)[:, None],
        "ident": np.eye(128, dtype=np.float32),
        "A0s": A0s, "A2": np.ascontiguousarray(A2),
        "W1h": W1h, "U": np.ascontiguousarray(U),
        "W1d": W1d, "b1f": b1f[:, None],
        "sew1": np.concatenate([se_w1[0:10], np.zeros((22, 64), np.float32),
                                se_w1[10:20]], 0),
        "seb1": se_b1[:, None],
        "cxw2": f32(inputs["cx_w2"]), "cxb2": f32(inputs["cx_b2"])[:, None],
        "cxw3": f32(inputs["cx_w3"]), "cxb3": f32(inputs["cx_b3"])[:, None],
        "atw1": f32(inputs["atc_w1"]), "atb1": f32(inputs["atc_b1"])[:, None],
        "atw2": f32(inputs["atc_w2"]), "atb2": f32(inputs["atc_b2"])[:, None],
        "W1cd": W1cd, "abp": abp, "W2blk": W2blk, "b2q": b2q,
        "w3blk": w3blk, "Bm1": Bm1,
    }
    consts = {k: np.ascontiguousarray(v, dtype=np.float32) for k, v in consts.items()}

    in_maps = []
    for i in range(NCORES):
        sl = slice(i * BC, (i + 1) * BC)
        hci = hc[sl].astype(np.float32)
        m = dict(consts)
        m["ecT"] = np.ascontiguousarray(ec[sl].astype(np.float32)[None, :])
        m["gsT"] = np.ascontiguousarray(gs[sl].T)
        m["dcT"] = np.ascontiguousarray(dc[sl].T)
        m["hcS"] = np.ascontiguousarray(
            hci.reshape(16, 128, 8).transpose(1, 0, 2).reshape(128, 128))
        m["hsBM"] = np.ascontiguousarray(
            hs[sl].astype(np.float32).reshape(16, 128).T)
        in_maps.append(m)
    return in_maps, consts


# ---------------------------------------------------------------------------
# device program
# ---------------------------------------------------------------------------

def _build(consts, n_act_evict=16, n_s2_act=8):
    import concourse.bass as bass
    import concourse.tile as tile
    import concourse.mybir as mybir
    from concourse import bacc

    dt = mybir.dt.float32
    AF = mybir.ActivationFunctionType
    OP = mybir.AluOpType
    AX = mybir.AxisListType

    nc = bacc.Bacc("TRN2", target_bir_lowering=False, debug=False,
                   enable_asserts=False, num_devices=NCORES)

    din = {}
    for name, arr in consts.items():
        din[name] = nc.dram_tensor(name, list(arr.shape), dt,
                                   kind="ExternalInput").ap()
    for name, shape in (("ecT", [1, BC]), ("gsT", [10, BC]), ("dcT", [54, BC]),
                        ("hcS", [128, 128]), ("hsBM", [128, 16])):
        din[name] = nc.dram_tensor(name, shape, dt, kind="ExternalInput").ap()
    out_d = nc.dram_tensor("out", [BC, A], dt, kind="ExternalOutput").ap()
    # out rows b = 512*n + 128*s + p  ->  [n][p, s, a]
    out_r = out_d.rearrange("(n s p) a -> n p s a", n=NCH, s=4, p=128)

    with tile.TileContext(nc) as tc, ExitStack() as ctx:
        cpool = ctx.enter_context(tc.tile_pool(name="consts", bufs=1))
        core = ctx.enter_context(tc.tile_pool(name="core", bufs=1))
        work = ctx.enter_context(tc.tile_pool(name="work", bufs=2))
        s1p = ctx.enter_context(tc.tile_pool(name="s1p", bufs=8))
        s2p = ctx.enter_context(tc.tile_pool(name="s2p", bufs=4))
        fout = ctx.enter_context(tc.tile_pool(name="fout", bufs=2))
        ps_f = ctx.enter_context(tc.tile_pool(name="ps_f", bufs=2, space="PSUM"))
        ps_c = ctx.enter_context(tc.tile_pool(name="ps_c", bufs=2, space="PSUM"))
        ps_z = ctx.enter_context(tc.tile_pool(name="ps_z", bufs=2, space="PSUM"))
        ps_o = ctx.enter_context(tc.tile_pool(name="ps_o", bufs=2, space="PSUM"))

        # ---- constants into SBUF
        cs = {}
        for name, arr in consts.items():
            t = cpool.tile(list(arr.shape), dt, tag=name, name="c_" + name)
            nc.sync.dma_start(t[:], din[name])
            cs[name] = t

        # ---- core-level tiles
        sh_in = core.tile([44, BC], dt, tag="sh_in")    # strat_in rows + r row
        nc.vector.memset(sh_in[:, :], 0.0)
        nc.sync.dma_start(sh_in[0:10, :], din["gsT"])
        dcT = core.tile([54, BC], dt, tag="dcT")
        nc.sync.dma_start(dcT[:], din["dcT"])

        hcS = core.tile([128, 128], dt, tag="hcS")
        nc.sync.dma_start(hcS[:], din["hcS"])
        S = core.tile([128, 176], dt, tag="S")          # 11 blocks of 16
        nc.sync.dma_start(S[:, 0:16], din["hsBM"])
        expl = core.tile([4, BC], dt, tag="expl")

        # ---- per-card features (GPSIMD, batch-major, exact integer ops)
        ft = {k: core.tile([128, 128], dt, tag=f"ft_{k}", name=f"ft_{k}") for k in
              ("t", "g13", "g26", "g39", "s0", "m13", "v0", "mask",
               "s0p", "ace", "face", "lowd", "low", "su1", "su2", "su3", "su4")}
        g = nc.gpsimd
        g.tensor_scalar(ft["t"][:], hcS[:], -1.0, None, OP.add)
        g.tensor_scalar(ft["g13"][:], ft["t"][:], 13.0, None, OP.is_ge)
        g.tensor_scalar(ft["g26"][:], ft["t"][:], 26.0, None, OP.is_ge)
        g.tensor_scalar(ft["g39"][:], ft["t"][:], 39.0, None, OP.is_ge)
        g.tensor_tensor(ft["s0"][:], ft["g13"][:], ft["g26"][:], OP.add)
        g.tensor_tensor(ft["s0"][:], ft["s0"][:], ft["g39"][:], OP.add)
        g.tensor_scalar(ft["m13"][:], ft["s0"][:], 13.0, None, OP.mult)
        g.tensor_tensor(ft["v0"][:], ft["t"][:], ft["m13"][:], OP.subtract)
        g.tensor_scalar(ft["mask"][:], hcS[:], 0.5, None, OP.is_ge)
        g.tensor_scalar(ft["s0p"][:], ft["s0"][:], 1.0, None, OP.add)
        g.tensor_tensor(ft["s0p"][:], ft["s0p"][:], ft["mask"][:], OP.mult)
        g.tensor_scalar(ft["ace"][:], ft["v0"][:], 0.0, None, OP.is_equal)
        g.tensor_scalar(ft["face"][:], ft["v0"][:], 10.0, None, OP.is_ge)
        g.tensor_scalar(ft["lowd"][:], ft["v0"][:], 1.0, None, OP.is_ge)
        g.tensor_scalar(ft["low"][:], ft["v0"][:], 5.0, None, OP.is_le)
        g.tensor_tensor(ft["low"][:], ft["low"][:], ft["lowd"][:], OP.mult)
        for k, s in (("su1", 1.0), ("su2", 2.0), ("su3", 3.0), ("su4", 4.0)):
            g.tensor_scalar(ft[k][:], ft["s0p"][:], s, None, OP.is_equal)

        # ---- reduce 8 cards -> per-batch sums into S blocks (DVE)
        v = nc.vector
        for blk, k in ((1, "ace"), (2, "face"), (3, "low"),
                       (4, "su1"), (5, "su2"), (6, "su3"), (7, "su4")):
            src = ft[k].rearrange("p (j c) -> p j c", c=8)
            v.tensor_reduce(S[:, 16 * blk:16 * blk + 16], src, AX.X, OP.add)

        # ---- hvr, sdiv, r (batch-major small tiles)
        hsr = core.tile([128, 16], dt, tag="hsr")
        v.tensor_scalar(hsr[:], S[:, 0:16], 1e-8, None, OP.add)
        v.reciprocal(hsr[:], hsr[:])
        v.tensor_tensor(S[:, 128:144], S[:, 32:48], hsr[:], OP.mult)  # hvr
        ge = [core.tile([128, 16], dt, tag=f"ge{k}", name=f"ge{k}") for k in range(4)]
        for k in range(4):
            v.tensor_scalar(ge[k][:], S[:, 64 + 16 * k:80 + 16 * k], 0.5, None,
                            OP.is_ge)
        v.tensor_tensor(ge[0][:], ge[0][:], ge[1][:], OP.add)
        v.tensor_tensor(ge[2][:], ge[2][:], ge[3][:], OP.add)
        v.tensor_tensor(S[:, 144:160], ge[0][:], ge[2][:], OP.add)   # sdiv cnt
        rmax = core.tile([128, 16], dt, tag="rmax")
        v.tensor_scalar(rmax[:], S[:, 0:16], 1.0, None, OP.max)
        v.reciprocal(S[:, 160:176], rmax[:])                          # r

        sc = nc.scalar
        iota_col = cs["iota54"][:, 0:1]

        for n in range(NCH):
            cols = slice(N * n, N * (n + 1))

            # -- rotate per-batch scalars into rows: 4 transposes of [128, 11]
            scalT = ps_f.tile([128, N], dt, tag="fe", name="scalT")
            S_kj = S.rearrange("p (k j) -> p j k", j=16)
            for s in range(4):
                nc.tensor.transpose(scalT[0:11, 128 * s:128 * (s + 1)],
                                    S_kj[:, 4 * n + s, :], cs["ident"][:])
            sc.activation(sh_in[32:43, cols], scalT[0:11, :], AF.Copy)

            # -- strat hidden layer
            shp = ps_f.tile([128, N], dt, tag="fe")
            nc.tensor.matmul(shp[0:64, :], cs["sew1"][:], sh_in[0:42, cols],
                             start=True, stop=True)
            sh = work.tile([64, N], dt, tag="sh")
            sc.activation(sh[:], shp[0:64, :], AF.Relu, bias=cs["seb1"][:, 0:1])

            # -- enemy one-hot (broadcast enemy-card row straight from DRAM)
            ecbc = work.tile([54, N], dt, tag="ecbc")
            ec_src = bass.AP(din["ecT"].tensor, din["ecT"].offset + N * n,
                             [[0, 54], [1, N]])
            nc.sync.dma_start(ecbc[:], ec_src)
            oh = work.tile([54, N], dt, tag="oh")
            v.tensor_scalar(oh[:], ecbc[:], iota_col, None, OP.is_equal)

            # -- hand_ctx = (A0s^T oh) * r
            rrow = work.tile([1, N], dt, tag="rrow")
            nc.sync.dma_start(rrow[:], sh_in[42:43, cols])
            r32 = work.tile([32, N], dt, tag="r32")
            nc.gpsimd.partition_broadcast(r32[:], rrow[:], channels=32)
            yps = ps_f.tile([128, N], dt, tag="fe")
            nc.tensor.matmul(yps[0:32, :], cs["A0s"][:], oh[:],
                             start=True, stop=True)
            hctx = work.tile([32, N], dt, tag="hctx")
            v.tensor_tensor(hctx[:], yps[0:32, :], r32[:], OP.mult)

            # -- z1 = A2^T oh + W1h^T hctx + U^T sh + W1d^T dcT + b1f
            z1 = ps_f.tile([128, N], dt, tag="fe")
            nc.tensor.matmul(z1[:], cs["A2"][:], oh[:], start=True, stop=False)
            nc.tensor.matmul(z1[:], cs["W1h"][:], hctx[:], start=False, stop=False)
            nc.tensor.matmul(z1[:], cs["U"][:], sh[:], start=False, stop=False)
            nc.tensor.matmul(z1[:], cs["W1d"][:], dcT[:, cols], start=False,
                             stop=True)
            h1 = work.tile([128, N], dt, tag="h1")
            sc.activation(h1[:], z1[:], AF.Relu, bias=cs["b1f"][:, 0:1])

            h2p = ps_f.tile([128, N], dt, tag="fe")
            nc.tensor.matmul(h2p[:], cs["cxw2"][:], h1[:], start=True, stop=True)
            h2 = work.tile([128, N], dt, tag="h2")
            sc.activation(h2[:], h2p[:], AF.Relu, bias=cs["cxb2"][:, 0:1])

            ctxp = ps_f.tile([128, N], dt, tag="fe")
            nc.tensor.matmul(ctxp[:], cs["cxw3"][:], h2[:], start=True, stop=True)
            ctxt = work.tile([128, N], dt, tag="ctxt")
            sc.activation(ctxt[:], ctxp[:], AF.Identity, bias=cs["cxb3"][:, 0:1])

            # -- action-type probs (unnormalized exp)
            tphp = ps_f.tile([128, N], dt, tag="fe")
            nc.tensor.matmul(tphp[0:64, :], cs["atw1"][:], ctxt[:],
                             start=True, stop=True)
            tph = work.tile([64, N], dt, tag="tph")
            sc.activation(tph[:], tphp[0:64, :], AF.Relu, bias=cs["atb1"][:, 0:1])
            tlp = ps_f.tile([128, N], dt, tag="fe")
            nc.tensor.matmul(tlp[0:4, :], cs["atw2"][:], tph[:],
                             start=True, stop=True)
            sc.activation(expl[:, cols], tlp[0:4, :], AF.Exp,
                          bias=cs["atb2"][:, 0:1])

            # -- action MLP
            ctx1d = ps_c.tile([128, N], dt, tag="ctx1d")
            nc.tensor.matmul(ctx1d[:], cs["W1cd"][:], ctxt[:], start=True,
                             stop=True)
            score = ps_o.tile([128, 128], dt, tag="fin")
            for q in range(8):
                s1pair = []
                for p in (2 * q, 2 * q + 1):
                    on_act = (p % 16) < n_act_evict
                    t1 = s1p.tile([128, N], dt, tag="s1a" if on_act else "s1v",
                                  name=f"s1_{n}_{p}", bufs=4)
                    if on_act:
                        sc.activation(t1[:], ctx1d[:], AF.Relu,
                                      bias=cs["abp"][:, p:p + 1])
                    else:
                        v.tensor_scalar(t1[:], ctx1d[:], cs["abp"][:, p:p + 1],
                                        0.0, OP.add, OP.max)
                    s1pair.append(t1)
                z2q = ps_z.tile([128, N], dt, tag="z2", name=f"z2_{n}_{q}")
                nc.tensor.matmul(z2q[0:64, :], cs["W2blk"][:], s1pair[0][:],
                                 start=True, stop=True)
                nc.tensor.matmul(z2q[64:128, :], cs["W2blk"][:],
                                 s1pair[1][:], start=True, stop=True)
                on_act2 = q < n_s2_act
                t = s2p.tile([128, N], dt, tag="s2a" if on_act2 else "s2v",
                             name=f"s2_{n}_{q}", bufs=3)
                if on_act2:
                    sc.activation(t[:], z2q[:], AF.Relu, bias=cs["b2q"][:, 0:1])
                else:
                    v.tensor_scalar(t[:], z2q[:], cs["b2q"][:, 0:1], 0.0,
                                    OP.add, OP.max)
                for s in range(4):
                    nc.tensor.matmul(score[:, 32 * s + 4 * q:32 * s + 4 * q + 4],
                                     t[:, 128 * s:128 * (s + 1)],
                                     cs["w3blk"][:], start=True, stop=True)

            numer = ps_o.tile([128, 132], dt, tag="fin")
            for s in range(4):
                nc.tensor.matmul(numer[:, 33 * s:33 * (s + 1)],
                                 expl[:, N * n + 128 * s:N * n + 128 * (s + 1)],
                                 cs["Bm1"][:], start=True, stop=True)

            recipT = fout.tile([128, 4], dt, tag="recip")
            den = numer.rearrange("p (s c) -> p s c", c=33)[:, :, 32]
            v.reciprocal(recipT[:], den)
            tmp = fout.tile([128, 120], dt, tag="tmp")
            for s in range(4):
                v.tensor_scalar(tmp[:, 30 * s:30 * (s + 1)],
                                numer[:, 33 * s:33 * s + 30],
                                recipT[:, s:s + 1], None, OP.mult)
            outT = fout.tile([128, 120], dt, tag="outT")
            sc_ap = score.rearrange("p (s c) -> p s c", c=32)[:, :, 0:30]
            v.tensor_tensor(outT.rearrange("p (s c) -> p s c", c=30),
                            tmp.rearrange("p (s c) -> p s c", c=30),
                            sc_ap, OP.add)
            nc.sync.dma_start(out_r[n],
                              outT.rearrange("p (s c) -> p s c", c=30))

    nc.compile()
    return nc


def _get_program(consts):
    key = "prog"
    if key not in _cache:
        _cache[key] = _build(consts)
    return _cache[key]


def kernel(**inputs):
    in_maps, consts = _prep(inputs)
    nc = _get_program(consts)
    from concourse.bass_utils import run_bass_kernel_spmd
    res = run_bass_kernel_spmd(nc, in_maps, core_ids=list(range(NCORES)))
    out = np.concatenate([res.results[i]["out"] for i in range(NCORES)], 0)
    return out.astype(np.float32)


# revision 17
# speedup vs baseline: 1.5784x; 1.5784x over previous
"""Trainium2 Bass kernel for nn_EnhancedCardAwarePolicy.

Strategy: pure data-parallel across 8 NeuronCores (batch 16384 -> 2048/core).

Key algebraic simplifications (exactly value-preserving vs the reference):
  * The hand self-attention MHA is dead code: the cross-attention that
    consumes it has sequence length 1, so its softmax is identically 1 and
    its output is independent of the query.  hand_ctx reduces to
        (8 / max(hand_size,1)) * (enemy_emb @ he_wv @ he_wo + he_bv @ he_wo + he_bo)
  * Card encodings are pure functions of the card index 0..53 -> fold the
    embedding tables into one [54, 32] table, and fold that table through
    the downstream linear layers, so the enemy-card path becomes two
    matmuls against a one-hot [54, B] matrix built on-device.
  * strat_ctx's second linear layer is folded into cx_w1.
  * The per-action tables ([30,4] action_card_indices, replicated per the
    sharding hint) are folded into per-action bias vectors on the host.
  * softmax+bonus is computed unnormalized: out = score + (expl@Bm)/(expl@1),
    with the scalar bias as_b3 folded so one reciprocal+mul suffices.

Device layout: feature-major [D, B] activations; per-batch card features are
computed batch-major [128, 16] on GPSIMD and rotated into rows with PE
transposes.  The dominant cost (the [B, 30, 64/32] action MLP) runs as
2-actions-per-128-partition matmuls with relu+bias fused into the PSUM->SBUF
eviction, split across the Scalar and Vector engines.
"""

import numpy as np
from contextlib import ExitStack

B = 16384
NCORES = 8
BC = B // NCORES          # 2048 batch rows per core
NCH = 4                   # chunks per core
N = BC // NCH             # 512 batch columns per chunk
A = 30                    # real actions
AP_ = 32                  # padded actions
E = 32
HID = 128

_cache = {}


# ---------------------------------------------------------------------------
# host-side folding
# ---------------------------------------------------------------------------

def _card_table(val_emb, suit_emb, type_emb):
    """[54, 32] full card encoding table, matching _encode_cards."""
    c = np.arange(54)
    invalid = (c == 0) | (c == 53)
    v = np.where(invalid, 0, (c - 1) % 13 + 1)
    s = np.where(invalid, 0, (c - 1) // 13 + 1)
    ce = np.concatenate([val_emb[v], suit_emb[s]], axis=-1)          # [54, 32]
    ct = np.where(v == 11, 1, np.where(v == 12, 2, np.where(v == 13, 3, 0)))
    te = type_emb[ct]                                                # [54, 8]
    pad = np.zeros((54, E - te.shape[-1]), np.float32)
    return (ce + np.concatenate([te, pad], axis=-1)).astype(np.float32)


def _action_fold(ac, card_emb, ce_w1, ce_b1, ce_w2, ce_b2,
                 as_w1, as_b1, as_b3):
    """Per-action biases + bonus matrix from action_card_indices [30, 4]."""
    ac = np.asarray(ac, np.int64)
    mask = ac != 0
    combo_size = mask.sum(1).astype(np.float32)
    values = np.where(mask, (ac - 1) % 13 + 1, 0)
    has_valid = mask.any(1)
    fidx = np.argmax(mask, axis=1)
    fv = values[np.arange(ac.shape[0]), fidx]
    same = np.where(mask, values == fv[:, None], True).all(1).astype(np.float32)
    vf = values.astype(np.float32)
    attack = np.where(values == 1, 1.0,
             np.where(values == 11, 10.0,
             np.where(values == 12, 15.0,
             np.where(values == 13, 20.0, vf))))
    total = (attack * mask).sum(1).astype(np.float32)
    suits = np.where(mask, (ac - 1) // 13 + 1, 0)
    uniq = sum((suits == s).any(1) for s in (1, 2, 3, 4)).astype(np.float32)
    ace = ((values == 1) & mask).any(1).astype(np.float32)
    valid = ((combo_size <= 4.0) & ((same > 0) | (ace > 0))).astype(np.float32)
    feats = np.stack([combo_size, same, total, uniq, ace, valid], 1)
    feats = np.where(has_valid[:, None], feats, 0.0).astype(np.float32)

    emb = card_emb[ac]                                   # [30, 4, 32]
    m = mask.astype(np.float32)[..., None]
    cnt = np.maximum(m.sum(1), 1.0)
    act_emb = (emb * m).sum(1) / cnt
    act_emb = np.where(has_valid[:, None], act_emb, 0.0).astype(np.float32)
    combo_enc = np.maximum(feats @ ce_w1 + ce_b1, 0.0) @ ce_w2 + ce_b2

    action_bias = act_emb @ as_w1[HID:HID + E] + combo_enc @ as_w1[HID + E:] + as_b1

    strength = feats[:, 2] / 20.0
    b3 = float(as_b3[0])
    Bm1 = np.zeros((4, AP_ + 2), np.float32)
    for a in range(A):
        if has_valid[a]:
            col = np.array([strength[a], 1.0 - strength[a], 0.0, 0.0])
        else:
            col = np.array([0.0, 0.0, 0.0, 2.0])
        Bm1[:, a] = col + b3
    Bm1[:, AP_] = 1.0                                    # denominator column
    ab = np.zeros((AP_, 64), np.float32)
    ab[:A] = action_bias
    return ab, Bm1


def _prep(inputs):
    """Fold weights, build per-core input maps. Returns (in_maps, shapes)."""
    f32 = lambda x: np.ascontiguousarray(np.asarray(x), dtype=np.float32)
    hc = np.asarray(inputs["hand_cards"])        # [B, 8] int
    ec = np.asarray(inputs["enemy_card"])        # [B]
    hs = np.asarray(inputs["hand_size"])         # [B]
    gs = f32(inputs["game_state"])               # [B, 10]
    dc = f32(inputs["discard_pile_cards"])       # [B, 54]

    card_emb = _card_table(f32(inputs["val_emb"]), f32(inputs["suit_emb"]),
                           f32(inputs["type_emb"]))
    card_emb1 = np.concatenate([card_emb, np.ones((54, 1), np.float32)], 1)

    he_wv, he_bv = f32(inputs["he_wv"]), f32(inputs["he_bv"])
    he_wo, he_bo = f32(inputs["he_wo"]), f32(inputs["he_bo"])
    Mc = np.concatenate([he_wv @ he_wo, (he_bv @ he_wo + he_bo)[None]], 0)  # [33,32]
    A0s = 8.0 * (card_emb1 @ Mc)                                   # [54, 32]

    cx_w1, cx_b1 = f32(inputs["cx_w1"]), f32(inputs["cx_b1"])
    W1h = np.ascontiguousarray(cx_w1[0:E])                         # [32, 128]
    A2 = card_emb @ cx_w1[E:2 * E]                                 # [54, 128]
    W1s = cx_w1[2 * E:2 * E + 32]                                  # [32, 128]
    W1d = np.ascontiguousarray(cx_w1[2 * E + 32:])                 # [54, 128]
    se_w1, se_b1 = f32(inputs["se_w1"]).copy(), f32(inputs["se_b1"])
    se_w2, se_b2 = f32(inputs["se_w2"]), f32(inputs["se_b2"])
    U = se_w2 @ W1s                                                # [64, 128]
    b1f = cx_b1 + se_b2 @ W1s                                      # [128]
    se_w1[19] /= 4.0          # device computes suit-diversity count 0..4

    as_w1, as_b1 = f32(inputs["as_w1"]), f32(inputs["as_b1"])
    as_w2, as_b2 = f32(inputs["as_w2"]), f32(inputs["as_b2"])
    as_w3, as_b3 = f32(inputs["as_w3"]), f32(inputs["as_b3"])
    ab, Bm1 = _action_fold(inputs["action_card_indices"], card_emb,
                           f32(inputs["ce_w1"]), f32(inputs["ce_b1"]),
                           f32(inputs["ce_w2"]), f32(inputs["ce_b2"]),
                           as_w1, as_b1, as_b3)
    W1c = as_w1[:HID]                                              # [128, 64]
    W1cd = np.concatenate([W1c, W1c], 1)                           # [128, 128]
    abp = np.zeros((128, 16), np.float32)
    for p in range(16):
        abp[0:64, p] = ab[2 * p]
        abp[64:128, p] = ab[2 * p + 1]
    W2blk = np.zeros((128, 64), np.float32)
    W2blk[0:64, 0:32] = as_w2
    W2blk[64:128, 32:64] = as_w2
    b2q = np.tile(as_b2, 4)[:, None].astype(np.float32)            # [128, 1]
    w3blk = np.zeros((128, 4), np.float32)
    for i in range(4):
        w3blk[32 * i:32 * i + 32, i] = as_w3[:, 0]

    consts = {
        "iota54": np.arange(54, dtype=np.float32)[:, None],
        "ident": np.eye(128, dtype=np.float32),
        "A0s": A0s, "A2": np.ascontiguousarray(A2),
        "W1h": W1h, "U": np.ascontiguousarray(U),
        "W1d": W1d, "b1f": b1f[:, None],
        "sew1": np.concatenate([se_w1[0:10], np.zeros((22, 64), np.float32),
                                se_w1[10:20]], 0),
        "seb1": se_b1[:, None],
        "cxw2": f32(inputs["cx_w2"]), "cxb2": f32(inputs["cx_b2"])[:, None],
        "cxw3": f32(inputs["cx_w3"]), "cxb3": f32(inputs["cx_b3"])[:, None],
        "atw1": f32(inputs["atc_w1"]), "atb1": f32(inputs["atc_b1"])[:, None],
        "atw2": f32(inputs["atc_w2"]), "atb2": f32(inputs["atc_b2"])[:, None],
        "W1cd": W1cd, "abp": abp, "W2blk": W2blk, "b2q": b2q,
        "w3blk": w3blk, "Bm1": Bm1,
    }
    import ml_dtypes
    consts = {k: np.ascontiguousarray(v, dtype=np.float32) for k, v in consts.items()}
    for k in ("W2blk", "w3blk"):
        consts[k] = consts[k].astype(np.float16)

    in_maps = []
    for i in range(NCORES):
        sl = slice(i * BC, (i + 1) * BC)
        hci = hc[sl].astype(np.float32)
        m = dict(consts)
        m["ecT"] = np.ascontiguousarray(ec[sl].astype(np.float32)[None, :])
        gst = np.zeros((32, BC), np.float32)
        gst[0:10] = gs[sl].T
        m["gsT"] = gst
        m["dcT"] = np.ascontiguousarray(dc[sl].T)
        m["hcS"] = np.ascontiguousarray(
            hci.reshape(16, 128, 8).transpose(1, 0, 2).reshape(128, 128))
        m["hsBM"] = np.ascontiguousarray(
            hs[sl].astype(np.float32).reshape(16, 128).T)
        in_maps.append(m)
    return in_maps, consts


# ---------------------------------------------------------------------------
# device program
# ---------------------------------------------------------------------------

def _build(consts, n_act_evict=16, n_s2_act=8):
    import concourse.bass as bass
    import concourse.tile as tile
    import concourse.mybir as mybir
    from concourse import bacc

    dt = mybir.dt.float32
    dtb = mybir.dt.float16
    dtr = mybir.dt.float32r
    rr = lambda ap: ap.bitcast(dtr)
    AF = mybir.ActivationFunctionType
    OP = mybir.AluOpType
    AX = mybir.AxisListType

    nc = bacc.Bacc("TRN2", target_bir_lowering=False, debug=False,
                   enable_asserts=False, num_devices=NCORES)

    MM_CONSTS = {"sew1", "A0s", "A2", "W1h", "U", "W1d", "cxw2", "cxw3",
                 "atw1", "atw2", "W1cd", "Bm1"}
    BF_CONSTS = {"W2blk", "w3blk"}
    def cdt(name):
        return dtr if name in MM_CONSTS else (dtb if name in BF_CONSTS else dt)
    din = {}
    for name, arr in consts.items():
        din[name] = nc.dram_tensor(name, list(arr.shape), cdt(name),
                                   kind="ExternalInput").ap()
    for name, shape in (("ecT", [1, BC]), ("gsT", [32, BC]), ("dcT", [54, BC]),
                        ("hcS", [128, 128]), ("hsBM", [128, 16])):
        din[name] = nc.dram_tensor(name, shape,
                                   dtr if name in ("gsT", "dcT") else dt,
                                   kind="ExternalInput").ap()
    out_d = nc.dram_tensor("out", [BC, A], dt, kind="ExternalOutput").ap()
    # out rows b = 512*n + 128*s + p  ->  [n][p, s, a]
    out_r = out_d.rearrange("(n s p) a -> n p s a", n=NCH, s=4, p=128)

    with tile.TileContext(nc) as tc, ExitStack() as ctx:
        cpool = ctx.enter_context(tc.tile_pool(name="consts", bufs=1))
        core = ctx.enter_context(tc.tile_pool(name="core", bufs=1))
        work = ctx.enter_context(tc.tile_pool(name="work", bufs=2))
        s1p = ctx.enter_context(tc.tile_pool(name="s1p", bufs=8))
        s2p = ctx.enter_context(tc.tile_pool(name="s2p", bufs=4))
        fout = ctx.enter_context(tc.tile_pool(name="fout", bufs=2))
        ps_f = ctx.enter_context(tc.tile_pool(name="ps_f", bufs=2, space="PSUM"))
        ps_c = ctx.enter_context(tc.tile_pool(name="ps_c", bufs=2, space="PSUM"))
        ps_z = ctx.enter_context(tc.tile_pool(name="ps_z", bufs=2, space="PSUM"))
        ps_o = ctx.enter_context(tc.tile_pool(name="ps_o", bufs=2, space="PSUM"))

        # ---- constants into SBUF
        cs = {}
        for name, arr in consts.items():
            t = cpool.tile(list(arr.shape), cdt(name), tag=name, name="c_" + name)
            nc.sync.dma_start(t[:], din[name])
            cs[name] = t

        # ---- core-level tiles
        sh_in = core.tile([44, BC], dtr, tag="sh_in")    # strat_in rows + r row
        nc.sync.dma_start(sh_in[0:32, :], din["gsT"])
        dcT = core.tile([54, BC], dtr, tag="dcT")
        nc.sync.dma_start(dcT[:], din["dcT"])

        hcS = core.tile([128, 128], dt, tag="hcS")
        nc.sync.dma_start(hcS[:], din["hcS"])
        S = core.tile([128, 176], dt, tag="S")          # 11 blocks of 16
        nc.sync.dma_start(S[:, 0:16], din["hsBM"])
        expl = core.tile([4, BC], dtr, tag="expl")

        # ---- per-card features (GPSIMD, batch-major, exact integer ops)
        ft = {k: core.tile([128, 128], dt, tag=f"ft_{k}", name=f"ft_{k}") for k in
              ("t", "g13", "g26", "g39", "s0", "m13", "v0", "mask",
               "s0p", "ace", "face", "lowd", "low", "su1", "su2", "su3", "su4")}
        g = nc.gpsimd
        g.tensor_scalar(ft["t"][:], hcS[:], -1.0, None, OP.add)
        g.tensor_scalar(ft["g13"][:], ft["t"][:], 13.0, None, OP.is_ge)
        g.tensor_scalar(ft["g26"][:], ft["t"][:], 26.0, None, OP.is_ge)
        g.tensor_scalar(ft["g39"][:], ft["t"][:], 39.0, None, OP.is_ge)
        g.tensor_tensor(ft["s0"][:], ft["g13"][:], ft["g26"][:], OP.add)
        g.tensor_tensor(ft["s0"][:], ft["s0"][:], ft["g39"][:], OP.add)
        g.tensor_scalar(ft["m13"][:], ft["s0"][:], 13.0, None, OP.mult)
        g.tensor_tensor(ft["v0"][:], ft["t"][:], ft["m13"][:], OP.subtract)
        g.tensor_scalar(ft["mask"][:], hcS[:], 0.5, None, OP.is_ge)
        g.tensor_scalar(ft["s0p"][:], ft["s0"][:], 1.0, None, OP.add)
        g.tensor_tensor(ft["s0p"][:], ft["s0p"][:], ft["mask"][:], OP.mult)
        g.tensor_scalar(ft["ace"][:], ft["v0"][:], 0.0, None, OP.is_equal)
        g.tensor_scalar(ft["face"][:], ft["v0"][:], 10.0, None, OP.is_ge)
        g.tensor_scalar(ft["lowd"][:], ft["v0"][:], 1.0, None, OP.is_ge)
        g.tensor_scalar(ft["low"][:], ft["v0"][:], 5.0, None, OP.is_le)
        g.tensor_tensor(ft["low"][:], ft["low"][:], ft["lowd"][:], OP.mult)
        for k, s in (("su1", 1.0), ("su2", 2.0), ("su3", 3.0), ("su4", 4.0)):
            g.tensor_scalar(ft[k][:], ft["s0p"][:], s, None, OP.is_equal)

        # ---- reduce 8 cards -> per-batch sums into S blocks (DVE)
        v = nc.vector
        for blk, k in ((1, "ace"), (2, "face"), (3, "low"),
                       (4, "su1"), (5, "su2"), (6, "su3"), (7, "su4")):
            src = ft[k].rearrange("p (j c) -> p j c", c=8)
            v.tensor_reduce(S[:, 16 * blk:16 * blk + 16], src, AX.X, OP.add)

        # ---- hvr, sdiv, r (batch-major small tiles)
        hsr = core.tile([128, 16], dt, tag="hsr")
        v.tensor_scalar(hsr[:], S[:, 0:16], 1e-8, None, OP.add)
        v.reciprocal(hsr[:], hsr[:])
        v.tensor_tensor(S[:, 128:144], S[:, 32:48], hsr[:], OP.mult)  # hvr
        ge = [core.tile([128, 16], dt, tag=f"ge{k}", name=f"ge{k}") for k in range(4)]
        for k in range(4):
            v.tensor_scalar(ge[k][:], S[:, 64 + 16 * k:80 + 16 * k], 0.5, None,
                            OP.is_ge)
        v.tensor_tensor(ge[0][:], ge[0][:], ge[1][:], OP.add)
        v.tensor_tensor(ge[2][:], ge[2][:], ge[3][:], OP.add)
        v.tensor_tensor(S[:, 144:160], ge[0][:], ge[2][:], OP.add)   # sdiv cnt
        rmax = core.tile([128, 16], dt, tag="rmax")
        v.tensor_scalar(rmax[:], S[:, 0:16], 1.0, None, OP.max)
        v.reciprocal(S[:, 160:176], rmax[:])                          # r

        sc = nc.scalar
        iota_col = cs["iota54"][:, 0:1]

        for n in range(NCH):
            cols = slice(N * n, N * (n + 1))

            # -- rotate per-batch scalars into rows: 4 transposes of [128, 11]
            scalT = ps_f.tile([128, N], dt, tag="fe", name="scalT")
            S_kj = S.rearrange("p (k j) -> p j k", j=16)
            for s in range(4):
                nc.tensor.transpose(scalT[0:11, 128 * s:128 * (s + 1)],
                                    S_kj[:, 4 * n + s, :], cs["ident"][:])
            sc.activation(sh_in[32:43, cols], scalT[0:11, :], AF.Copy)

            # -- strat hidden layer
            shp = ps_f.tile([128, N], dt, tag="fe")
            nc.tensor.matmul(shp[0:64, :], (cs["sew1"][:]), (sh_in[0:42, cols]),
                             start=True, stop=True)
            sh = work.tile([64, N], dtr, tag="sh")
            sc.activation(sh[:], shp[0:64, :], AF.Relu, bias=cs["seb1"][:, 0:1])

            # -- enemy one-hot (broadcast enemy-card row straight from DRAM)
            ecbc = work.tile([54, N], dt, tag="ecbc")
            ec_src = bass.AP(din["ecT"].tensor, din["ecT"].offset + N * n,
                             [[0, 54], [1, N]])
            nc.sync.dma_start(ecbc[:], ec_src)
            oh = work.tile([54, N], dtr, tag="oh")
            v.tensor_scalar(oh[:], ecbc[:], iota_col, None, OP.is_equal)

            # -- hand_ctx = (A0s^T oh) * r
            rrow = work.tile([1, N], dtr, tag="rrow")
            nc.sync.dma_start(rrow[:], sh_in[42:43, cols])
            r32 = work.tile([32, N], dtr, tag="r32")
            nc.gpsimd.partition_broadcast(r32[:], rrow[:], channels=32)
            yps = ps_f.tile([128, N], dt, tag="fe")
            nc.tensor.matmul(yps[0:32, :], (cs["A0s"][:]), (oh[:]),
                             start=True, stop=True)
            hctx = work.tile([32, N], dtr, tag="hctx")
            v.tensor_tensor(hctx[:], yps[0:32, :], r32[:], OP.mult)

            # -- z1 = A2^T oh + W1h^T hctx + U^T sh + W1d^T dcT + b1f
            z1 = ps_f.tile([128, N], dt, tag="fe")
            nc.tensor.matmul(z1[:], (cs["A2"][:]), (oh[:]), start=True, stop=False)
            nc.tensor.matmul(z1[:], (cs["W1h"][:]), (hctx[:]), start=False, stop=False)
            nc.tensor.matmul(z1[:], (cs["U"][:]), (sh[:]), start=False, stop=False)
            nc.tensor.matmul(z1[:], (cs["W1d"][:]), (dcT[:, cols]), start=False,
                             stop=True)
            h1 = work.tile([128, N], dtr, tag="h1")
            sc.activation(h1[:], z1[:], AF.Relu, bias=cs["b1f"][:, 0:1])

            h2p = ps_f.tile([128, N], dt, tag="fe")
            nc.tensor.matmul(h2p[:], (cs["cxw2"][:]), (h1[:]), start=True, stop=True)
            h2 = work.tile([128, N], dtr, tag="h2")
            sc.activation(h2[:], h2p[:], AF.Relu, bias=cs["cxb2"][:, 0:1])

            ctxp = ps_f.tile([128, N], dt, tag="fe")
            nc.tensor.matmul(ctxp[:], (cs["cxw3"][:]), (h2[:]), start=True, stop=True)
            ctxt = work.tile([128, N], dtr, tag="ctxt")
            sc.activation(ctxt[:], ctxp[:], AF.Identity, bias=cs["cxb3"][:, 0:1])

            # -- action-type probs (unnormalized exp)
            tphp = ps_f.tile([128, N], dt, tag="fe")
            nc.tensor.matmul(tphp[0:64, :], (cs["atw1"][:]), (ctxt[:]),
                             start=True, stop=True)
            tph = work.tile([64, N], dtr, tag="tph")
            sc.activation(tph[:], tphp[0:64, :], AF.Relu, bias=cs["atb1"][:, 0:1])
            tlp = ps_f.tile([128, N], dt, tag="fe")
            nc.tensor.matmul(tlp[0:4, :], (cs["atw2"][:]), (tph[:]),
                             start=True, stop=True)
            sc.activation(expl[:, cols], tlp[0:4, :], AF.Exp,
                          bias=cs["atb2"][:, 0:1])

            # -- action MLP
            ctx1d = ps_c.tile([128, N], dt, tag="ctx1d")
            nc.tensor.matmul(ctx1d[:], (cs["W1cd"][:]), (ctxt[:]), start=True,
                             stop=True)
            score = ps_o.tile([128, 128], dt, tag="fin")
            for q in range(8):
                s1pair = []
                for p in (2 * q, 2 * q + 1):
                    on_act = (p % 16) < n_act_evict
                    t1 = s1p.tile([128, N], dtb, tag="s1a" if on_act else "s1v",
                                  name=f"s1_{n}_{p}", bufs=4)
                    if on_act:
                        sc.activation(t1[:], ctx1d[:], AF.Relu,
                                      bias=cs["abp"][:, p:p + 1])
                    else:
                        v.tensor_scalar(t1[:], ctx1d[:], cs["abp"][:, p:p + 1],
                                        0.0, OP.add, OP.max)
                    s1pair.append(t1)
                z2q = ps_z.tile([128, N], dt, tag="z2", name=f"z2_{n}_{q}")
                nc.tensor.matmul(z2q[0:64, :], (cs["W2blk"][:]), (s1pair[0][:]),
                                 start=True, stop=True)
                nc.tensor.matmul(z2q[64:128, :], (cs["W2blk"][:]),
                                 (s1pair[1][:]), start=True, stop=True)
                on_act2 = q < n_s2_act
                t = s2p.tile([128, N], dtb, tag="s2a" if on_act2 else "s2v",
                             name=f"s2_{n}_{q}", bufs=3)
                if on_act2:
                    sc.activation(t[:], z2q[:], AF.Relu, bias=cs["b2q"][:, 0:1])
                else:
                    v.tensor_scalar(t[:], z2q[:], cs["b2q"][:, 0:1], 0.0,
                                    OP.add, OP.max)
                for s in range(4):
                    nc.tensor.matmul(score[:, 32 * s + 4 * q:32 * s + 4 * q + 4],
                                     (t[:, 128 * s:128 * (s + 1)]),
                                     (cs["w3blk"][:]), start=True, stop=True)

            numer = ps_o.tile([128, 136], dt, tag="fin")
            for s in range(4):
                nc.tensor.matmul(numer[:, 34 * s:34 * (s + 1)],
                                 (expl[:, N * n + 128 * s:N * n + 128 * (s + 1)]),
                                 (cs["Bm1"][:]), start=True, stop=True)

            recipT = fout.tile([128, 4], dt, tag="recip")
            den = numer.rearrange("p (s c) -> p s c", c=34)[:, :, 32]
            v.reciprocal(recipT[:], den)
            tmp = fout.tile([128, 120], dt, tag="tmp")
            for s in range(4):
                v.tensor_scalar(tmp[:, 30 * s:30 * (s + 1)],
                                numer[:, 34 * s:34 * s + 30],
                                recipT[:, s:s + 1], None, OP.mult)
            outT = fout.tile([128, 120], dt, tag="outT")
            sc_ap = score.rearrange("p (s c) -> p s c", c=32)[:, :, 0:30]
            v.tensor_tensor(outT.rearrange("p (s c) -> p s c", c=30),
                            tmp.rearrange("p (s c) -> p s c", c=30),
                            sc_ap, OP.add)
            nc.sync.dma_start(out_r[n],
                              outT.rearrange("p (s c) -> p s c", c=30))

    nc.compile()
    return nc


def _get_program(consts):
    key = "prog"
    if key not in _cache:
        _cache[key] = _build(consts)
    return _cache[key]


def kernel(**inputs):
    in_maps, consts = _prep(inputs)
    nc = _get_program(consts)
    from concourse.bass_utils import run_bass_kernel_spmd
    res = run_bass_kernel_spmd(nc, in_maps, core_ids=list(range(NCORES)))
    out = np.concatenate([res.results[i]["out"] for i in range(NCORES)], 0)
    return out.astype(np.float32)


# revision 18
# speedup vs baseline: 2.0747x; 1.3144x over previous
"""Trainium2 Bass kernel for nn_EnhancedCardAwarePolicy.

Strategy: pure data-parallel across 8 NeuronCores (batch 16384 -> 2048/core).

Key algebraic simplifications (exactly value-preserving vs the reference):
  * The hand self-attention MHA is dead code: the cross-attention that
    consumes it has sequence length 1, so its softmax is identically 1 and
    its output is independent of the query.  hand_ctx reduces to
        (8 / max(hand_size,1)) * (enemy_emb @ he_wv @ he_wo + he_bv @ he_wo + he_bo)
  * Card encodings are pure functions of the card index 0..53 -> fold the
    embedding tables into one [54, 32] table, and fold that table through
    the downstream linear layers, so the enemy-card path becomes two
    matmuls against a one-hot [54, B] matrix built on-device.
  * strat_ctx's second linear layer is folded into cx_w1.
  * The per-action tables ([30,4] action_card_indices, replicated per the
    sharding hint) are folded into per-action bias vectors on the host.
  * softmax+bonus is computed unnormalized: out = score + (expl@Bm)/(expl@1),
    with the scalar bias as_b3 folded so one reciprocal+mul suffices.

Device layout: feature-major [D, B] activations; per-batch card features are
computed batch-major [128, 16] on GPSIMD and rotated into rows with PE
transposes.  The dominant cost (the [B, 30, 64/32] action MLP) runs as
2-actions-per-128-partition matmuls with relu+bias fused into the PSUM->SBUF
eviction, split across the Scalar and Vector engines.
"""

import numpy as np
from contextlib import ExitStack

B = 16384
NCORES = 8
BC = B // NCORES          # 2048 batch rows per core
NCH = 4                   # chunks per core
N = BC // NCH             # 512 batch columns per chunk
A = 30                    # real actions
AP_ = 32                  # padded actions
E = 32
HID = 128

_cache = {}


# ---------------------------------------------------------------------------
# host-side folding
# ---------------------------------------------------------------------------

def _card_table(val_emb, suit_emb, type_emb):
    """[54, 32] full card encoding table, matching _encode_cards."""
    c = np.arange(54)
    invalid = (c == 0) | (c == 53)
    v = np.where(invalid, 0, (c - 1) % 13 + 1)
    s = np.where(invalid, 0, (c - 1) // 13 + 1)
    ce = np.concatenate([val_emb[v], suit_emb[s]], axis=-1)          # [54, 32]
    ct = np.where(v == 11, 1, np.where(v == 12, 2, np.where(v == 13, 3, 0)))
    te = type_emb[ct]                                                # [54, 8]
    pad = np.zeros((54, E - te.shape[-1]), np.float32)
    return (ce + np.concatenate([te, pad], axis=-1)).astype(np.float32)


def _action_fold(ac, card_emb, ce_w1, ce_b1, ce_w2, ce_b2,
                 as_w1, as_b1, as_b3):
    """Per-action biases + bonus matrix from action_card_indices [30, 4]."""
    ac = np.asarray(ac, np.int64)
    mask = ac != 0
    combo_size = mask.sum(1).astype(np.float32)
    values = np.where(mask, (ac - 1) % 13 + 1, 0)
    has_valid = mask.any(1)
    fidx = np.argmax(mask, axis=1)
    fv = values[np.arange(ac.shape[0]), fidx]
    same = np.where(mask, values == fv[:, None], True).all(1).astype(np.float32)
    vf = values.astype(np.float32)
    attack = np.where(values == 1, 1.0,
             np.where(values == 11, 10.0,
             np.where(values == 12, 15.0,
             np.where(values == 13, 20.0, vf))))
    total = (attack * mask).sum(1).astype(np.float32)
    suits = np.where(mask, (ac - 1) // 13 + 1, 0)
    uniq = sum((suits == s).any(1) for s in (1, 2, 3, 4)).astype(np.float32)
    ace = ((values == 1) & mask).any(1).astype(np.float32)
    valid = ((combo_size <= 4.0) & ((same > 0) | (ace > 0))).astype(np.float32)
    feats = np.stack([combo_size, same, total, uniq, ace, valid], 1)
    feats = np.where(has_valid[:, None], feats, 0.0).astype(np.float32)

    emb = card_emb[ac]                                   # [30, 4, 32]
    m = mask.astype(np.float32)[..., None]
    cnt = np.maximum(m.sum(1), 1.0)
    act_emb = (emb * m).sum(1) / cnt
    act_emb = np.where(has_valid[:, None], act_emb, 0.0).astype(np.float32)
    combo_enc = np.maximum(feats @ ce_w1 + ce_b1, 0.0) @ ce_w2 + ce_b2

    action_bias = act_emb @ as_w1[HID:HID + E] + combo_enc @ as_w1[HID + E:] + as_b1

    strength = feats[:, 2] / 20.0
    b3 = float(as_b3[0])
    Bm1 = np.zeros((4, AP_ + 2), np.float32)
    for a in range(A):
        if has_valid[a]:
            col = np.array([strength[a], 1.0 - strength[a], 0.0, 0.0])
        else:
            col = np.array([0.0, 0.0, 0.0, 2.0])
        Bm1[:, a] = col + b3
    Bm1[:, AP_] = 1.0                                    # denominator column
    ab = np.zeros((AP_, 64), np.float32)
    ab[:A] = action_bias
    return ab, Bm1


def _prep(inputs):
    """Fold weights, build per-core input maps. Returns (in_maps, shapes)."""
    f32 = lambda x: np.ascontiguousarray(np.asarray(x), dtype=np.float32)
    hc = np.asarray(inputs["hand_cards"])        # [B, 8] int
    ec = np.asarray(inputs["enemy_card"])        # [B]
    hs = np.asarray(inputs["hand_size"])         # [B]
    gs = f32(inputs["game_state"])               # [B, 10]
    dc = f32(inputs["discard_pile_cards"])       # [B, 54]

    card_emb = _card_table(f32(inputs["val_emb"]), f32(inputs["suit_emb"]),
                           f32(inputs["type_emb"]))
    card_emb1 = np.concatenate([card_emb, np.ones((54, 1), np.float32)], 1)

    he_wv, he_bv = f32(inputs["he_wv"]), f32(inputs["he_bv"])
    he_wo, he_bo = f32(inputs["he_wo"]), f32(inputs["he_bo"])
    Mc = np.concatenate([he_wv @ he_wo, (he_bv @ he_wo + he_bo)[None]], 0)  # [33,32]
    A0s = 8.0 * (card_emb1 @ Mc)                                   # [54, 32]

    cx_w1, cx_b1 = f32(inputs["cx_w1"]), f32(inputs["cx_b1"])
    W1h = np.ascontiguousarray(cx_w1[0:E])                         # [32, 128]
    A2 = card_emb @ cx_w1[E:2 * E]                                 # [54, 128]
    W1s = cx_w1[2 * E:2 * E + 32]                                  # [32, 128]
    W1d = np.ascontiguousarray(cx_w1[2 * E + 32:])                 # [54, 128]
    se_w1, se_b1 = f32(inputs["se_w1"]).copy(), f32(inputs["se_b1"])
    se_w2, se_b2 = f32(inputs["se_w2"]), f32(inputs["se_b2"])
    U = se_w2 @ W1s                                                # [64, 128]
    b1f = cx_b1 + se_b2 @ W1s                                      # [128]
    se_w1[19] /= 4.0          # device computes suit-diversity count 0..4

    as_w1, as_b1 = f32(inputs["as_w1"]), f32(inputs["as_b1"])
    as_w2, as_b2 = f32(inputs["as_w2"]), f32(inputs["as_b2"])
    as_w3, as_b3 = f32(inputs["as_w3"]), f32(inputs["as_b3"])
    ab, Bm1 = _action_fold(inputs["action_card_indices"], card_emb,
                           f32(inputs["ce_w1"]), f32(inputs["ce_b1"]),
                           f32(inputs["ce_w2"]), f32(inputs["ce_b2"]),
                           as_w1, as_b1, as_b3)
    W1c = as_w1[:HID]                                              # [128, 64]
    W1cd = np.concatenate([W1c, W1c], 1)                           # [128, 128]
    abp = np.zeros((128, 16), np.float32)
    for p in range(16):
        abp[0:64, p] = ab[2 * p]
        abp[64:128, p] = ab[2 * p + 1]
    W2blk = np.zeros((128, 64), np.float32)
    W2blk[0:64, 0:32] = as_w2
    W2blk[64:128, 32:64] = as_w2
    b2q = np.tile(as_b2, 4)[:, None].astype(np.float32)            # [128, 1]
    w3blk = np.zeros((128, 4), np.float32)
    for i in range(4):
        w3blk[32 * i:32 * i + 32, i] = as_w3[:, 0]

    consts = {
        "iota54": np.arange(54, dtype=np.float32)[:, None],
        "ident": np.eye(128, dtype=np.float32),
        "A0s": A0s, "A2": np.ascontiguousarray(A2),
        "W1h": W1h, "U": np.ascontiguousarray(U),
        "W1d": W1d, "b1f": b1f[:, None],
        "sew1": np.concatenate([se_w1[0:10], np.zeros((22, 64), np.float32),
                                se_w1[10:20]], 0),
        "seb1": se_b1[:, None],
        "cxw2": f32(inputs["cx_w2"]), "cxb2": f32(inputs["cx_b2"])[:, None],
        "cxw3": f32(inputs["cx_w3"]), "cxb3": f32(inputs["cx_b3"])[:, None],
        "atw1": f32(inputs["atc_w1"]), "atb1": f32(inputs["atc_b1"])[:, None],
        "atw2": f32(inputs["atc_w2"]), "atb2": f32(inputs["atc_b2"])[:, None],
        "W1cd": W1cd, "abp": abp, "W2blk": W2blk, "b2q": b2q,
        "w3blk": w3blk, "Bm1": Bm1,
    }
    import ml_dtypes
    consts = {k: np.ascontiguousarray(v, dtype=np.float32) for k, v in consts.items()}
    for k in ("W2blk", "w3blk"):
        consts[k] = consts[k].astype(np.float16)

    in_maps = []
    for i in range(NCORES):
        sl = slice(i * BC, (i + 1) * BC)
        hci = hc[sl].astype(np.float32)
        m = dict(consts)
        m["ecT"] = np.ascontiguousarray(ec[sl].astype(np.float32)[None, :])
        gst = np.zeros((32, BC), np.float32)
        gst[0:10] = gs[sl].T
        m["gsT"] = gst
        m["dcT"] = np.ascontiguousarray(dc[sl].T)
        m["hcS"] = np.ascontiguousarray(
            hci.reshape(16, 128, 8).transpose(1, 0, 2).reshape(128, 128))
        m["hsBM"] = np.ascontiguousarray(
            hs[sl].astype(np.float32).reshape(16, 128).T)
        in_maps.append(m)
    return in_maps, consts


# ---------------------------------------------------------------------------
# device program
# ---------------------------------------------------------------------------

def _build(consts, n_act_evict=11, n_s2_act=6):
    import concourse.bass as bass
    import concourse.tile as tile
    import concourse.mybir as mybir
    from concourse import bacc

    dt = mybir.dt.float32
    dtb = mybir.dt.float16
    dtr = mybir.dt.float32r
    rr = lambda ap: ap.bitcast(dtr)
    AF = mybir.ActivationFunctionType
    OP = mybir.AluOpType
    AX = mybir.AxisListType

    nc = bacc.Bacc("TRN2", target_bir_lowering=False, debug=False,
                   enable_asserts=False, num_devices=NCORES)

    MM_CONSTS = {"sew1", "A0s", "A2", "W1h", "U", "W1d", "cxw2", "cxw3",
                 "atw1", "atw2", "W1cd", "Bm1"}
    BF_CONSTS = {"W2blk", "w3blk"}
    def cdt(name):
        return dtr if name in MM_CONSTS else (dtb if name in BF_CONSTS else dt)
    din = {}
    for name, arr in consts.items():
        din[name] = nc.dram_tensor(name, list(arr.shape), cdt(name),
                                   kind="ExternalInput").ap()
    for name, shape in (("ecT", [1, BC]), ("gsT", [32, BC]), ("dcT", [54, BC]),
                        ("hcS", [128, 128]), ("hsBM", [128, 16])):
        din[name] = nc.dram_tensor(name, shape,
                                   dtr if name in ("gsT", "dcT") else dt,
                                   kind="ExternalInput").ap()
    out_d = nc.dram_tensor("out", [BC, A], dt, kind="ExternalOutput").ap()
    # out rows b = 512*n + 128*s + p  ->  [n][p, s, a]
    out_r = out_d.rearrange("(n s p) a -> n p s a", n=NCH, s=4, p=128)

    with tile.TileContext(nc) as tc, ExitStack() as ctx:
        cpool = ctx.enter_context(tc.tile_pool(name="consts", bufs=1))
        core = ctx.enter_context(tc.tile_pool(name="core", bufs=1))
        work = ctx.enter_context(tc.tile_pool(name="work", bufs=3))
        s1p = ctx.enter_context(tc.tile_pool(name="s1p", bufs=8))
        s2p = ctx.enter_context(tc.tile_pool(name="s2p", bufs=4))
        fout = ctx.enter_context(tc.tile_pool(name="fout", bufs=2))
        ps_f = ctx.enter_context(tc.tile_pool(name="ps_f", bufs=2, space="PSUM"))
        ps_c = ctx.enter_context(tc.tile_pool(name="ps_c", bufs=2, space="PSUM"))
        ps_z = ctx.enter_context(tc.tile_pool(name="ps_z", bufs=2, space="PSUM"))
        ps_o = ctx.enter_context(tc.tile_pool(name="ps_o", bufs=2, space="PSUM"))

        # ---- constants into SBUF
        cs = {}
        for name, arr in consts.items():
            t = cpool.tile(list(arr.shape), cdt(name), tag=name, name="c_" + name)
            nc.sync.dma_start(t[:], din[name])
            cs[name] = t

        # ---- core-level tiles
        sh_in = core.tile([44, BC], dtr, tag="sh_in")    # strat_in rows + r row
        nc.sync.dma_start(sh_in[0:32, :], din["gsT"])
        dcT = core.tile([54, BC], dtr, tag="dcT")
        nc.sync.dma_start(dcT[:], din["dcT"])

        hcS = core.tile([128, 128], dt, tag="hcS")
        nc.sync.dma_start(hcS[:], din["hcS"])
        S = core.tile([128, 176], dt, tag="S")          # 11 blocks of 16
        nc.sync.dma_start(S[:, 0:16], din["hsBM"])
        expl = core.tile([4, BC], dtr, tag="expl")

        # ---- per-card features (GPSIMD, batch-major, exact integer ops)
        ft = {k: core.tile([128, 128], dt, tag=f"ft_{k}", name=f"ft_{k}") for k in
              ("t", "g13", "g26", "g39", "s0", "m13", "v0", "mask",
               "s0p", "ace", "face", "lowd", "low", "su1", "su2", "su3", "su4")}
        g = nc.vector
        g.tensor_scalar(ft["t"][:], hcS[:], -1.0, None, OP.add)
        g.tensor_scalar(ft["g13"][:], ft["t"][:], 13.0, None, OP.is_ge)
        g.tensor_scalar(ft["g26"][:], ft["t"][:], 26.0, None, OP.is_ge)
        g.tensor_scalar(ft["g39"][:], ft["t"][:], 39.0, None, OP.is_ge)
        g.tensor_tensor(ft["s0"][:], ft["g13"][:], ft["g26"][:], OP.add)
        g.tensor_tensor(ft["s0"][:], ft["s0"][:], ft["g39"][:], OP.add)
        g.tensor_scalar(ft["m13"][:], ft["s0"][:], 13.0, None, OP.mult)
        g.tensor_tensor(ft["v0"][:], ft["t"][:], ft["m13"][:], OP.subtract)
        g.tensor_scalar(ft["mask"][:], hcS[:], 0.5, None, OP.is_ge)
        g.tensor_scalar(ft["s0p"][:], ft["s0"][:], 1.0, None, OP.add)
        g.tensor_tensor(ft["s0p"][:], ft["s0p"][:], ft["mask"][:], OP.mult)
        g.tensor_scalar(ft["ace"][:], ft["v0"][:], 0.0, None, OP.is_equal)
        g.tensor_scalar(ft["face"][:], ft["v0"][:], 10.0, None, OP.is_ge)
        g.tensor_scalar(ft["lowd"][:], ft["v0"][:], 1.0, None, OP.is_ge)
        g.tensor_scalar(ft["low"][:], ft["v0"][:], 5.0, None, OP.is_le)
        g.tensor_tensor(ft["low"][:], ft["low"][:], ft["lowd"][:], OP.mult)
        for k, s in (("su1", 1.0), ("su2", 2.0), ("su3", 3.0), ("su4", 4.0)):
            g.tensor_scalar(ft[k][:], ft["s0p"][:], s, None, OP.is_equal)

        # ---- reduce 8 cards -> per-batch sums into S blocks (DVE)
        v = nc.vector
        for blk, k in ((1, "ace"), (2, "face"), (3, "low"),
                       (4, "su1"), (5, "su2"), (6, "su3"), (7, "su4")):
            src = ft[k].rearrange("p (j c) -> p j c", c=8)
            v.tensor_reduce(S[:, 16 * blk:16 * blk + 16], src, AX.X, OP.add)

        # ---- hvr, sdiv, r (batch-major small tiles)
        hsr = core.tile([128, 16], dt, tag="hsr")
        v.tensor_scalar(hsr[:], S[:, 0:16], 1e-8, None, OP.add)
        v.reciprocal(hsr[:], hsr[:])
        v.tensor_tensor(S[:, 128:144], S[:, 32:48], hsr[:], OP.mult)  # hvr
        ge = [core.tile([128, 16], dt, tag=f"ge{k}", name=f"ge{k}") for k in range(4)]
        for k in range(4):
            v.tensor_scalar(ge[k][:], S[:, 64 + 16 * k:80 + 16 * k], 0.5, None,
                            OP.is_ge)
        v.tensor_tensor(ge[0][:], ge[0][:], ge[1][:], OP.add)
        v.tensor_tensor(ge[2][:], ge[2][:], ge[3][:], OP.add)
        v.tensor_tensor(S[:, 144:160], ge[0][:], ge[2][:], OP.add)   # sdiv cnt
        rmax = core.tile([128, 16], dt, tag="rmax")
        v.tensor_scalar(rmax[:], S[:, 0:16], 1.0, None, OP.max)
        v.reciprocal(S[:, 160:176], rmax[:])                          # r

        sc = nc.scalar
        iota_col = cs["iota54"][:, 0:1]

        for n in range(NCH):
            cols = slice(N * n, N * (n + 1))

            # -- rotate per-batch scalars into rows: 4 transposes of [128, 11]
            scalT = ps_f.tile([128, N], dt, tag="fe", name="scalT")
            S_kj = S.rearrange("p (k j) -> p j k", j=16)
            for s in range(4):
                nc.tensor.transpose(scalT[0:11, 128 * s:128 * (s + 1)],
                                    S_kj[:, 4 * n + s, :], cs["ident"][:])
            sc.activation(sh_in[32:43, cols], scalT[0:11, :], AF.Copy)

            # -- strat hidden layer
            shp = ps_f.tile([128, N], dt, tag="fe")
            nc.tensor.matmul(shp[0:64, :], (cs["sew1"][:]), (sh_in[0:42, cols]),
                             start=True, stop=True)
            sh = work.tile([64, N], dtr, tag="sh")
            sc.activation(sh[:], shp[0:64, :], AF.Relu, bias=cs["seb1"][:, 0:1])

            # -- enemy one-hot (broadcast enemy-card row straight from DRAM)
            ecbc = work.tile([54, N], dt, tag="ecbc")
            ec_src = bass.AP(din["ecT"].tensor, din["ecT"].offset + N * n,
                             [[0, 54], [1, N]])
            nc.sync.dma_start(ecbc[:], ec_src)
            oh = work.tile([54, N], dtr, tag="oh")
            v.tensor_scalar(oh[:], ecbc[:], iota_col, None, OP.is_equal)

            # -- hand_ctx = (A0s^T oh) * r
            rrow = work.tile([1, N], dtr, tag="rrow")
            nc.sync.dma_start(rrow[:], sh_in[42:43, cols])
            r32 = work.tile([32, N], dtr, tag="r32")
            nc.gpsimd.partition_broadcast(r32[:], rrow[:], channels=32)
            yps = ps_f.tile([128, N], dt, tag="fe")
            nc.tensor.matmul(yps[0:32, :], (cs["A0s"][:]), (oh[:]),
                             start=True, stop=True)
            hctx = work.tile([32, N], dtr, tag="hctx")
            v.tensor_tensor(hctx[:], yps[0:32, :], r32[:], OP.mult)

            # -- z1 = A2^T oh + W1h^T hctx + U^T sh + W1d^T dcT + b1f
            z1 = ps_f.tile([128, N], dt, tag="fe")
            nc.tensor.matmul(z1[:], (cs["A2"][:]), (oh[:]), start=True, stop=False)
            nc.tensor.matmul(z1[:], (cs["W1h"][:]), (hctx[:]), start=False, stop=False)
            nc.tensor.matmul(z1[:], (cs["U"][:]), (sh[:]), start=False, stop=False)
            nc.tensor.matmul(z1[:], (cs["W1d"][:]), (dcT[:, cols]), start=False,
                             stop=True)
            h1 = work.tile([128, N], dtr, tag="h1")
            sc.activation(h1[:], z1[:], AF.Relu, bias=cs["b1f"][:, 0:1])

            h2p = ps_f.tile([128, N], dt, tag="fe")
            nc.tensor.matmul(h2p[:], (cs["cxw2"][:]), (h1[:]), start=True, stop=True)
            h2 = work.tile([128, N], dtr, tag="h2")
            sc.activation(h2[:], h2p[:], AF.Relu, bias=cs["cxb2"][:, 0:1])

            ctxp = ps_f.tile([128, N], dt, tag="fe")
            nc.tensor.matmul(ctxp[:], (cs["cxw3"][:]), (h2[:]), start=True, stop=True)
            ctxt = work.tile([128, N], dtr, tag="ctxt")
            sc.activation(ctxt[:], ctxp[:], AF.Identity, bias=cs["cxb3"][:, 0:1])

            # -- action-type probs (unnormalized exp)
            tphp = ps_f.tile([128, N], dt, tag="fe")
            nc.tensor.matmul(tphp[0:64, :], (cs["atw1"][:]), (ctxt[:]),
                             start=True, stop=True)
            tph = work.tile([64, N], dtr, tag="tph")
            sc.activation(tph[:], tphp[0:64, :], AF.Relu, bias=cs["atb1"][:, 0:1])
            tlp = ps_f.tile([128, N], dt, tag="fe")
            nc.tensor.matmul(tlp[0:4, :], (cs["atw2"][:]), (tph[:]),
                             start=True, stop=True)
            sc.activation(expl[:, cols], tlp[0:4, :], AF.Exp,
                          bias=cs["atb2"][:, 0:1])

            # -- action MLP
            ctx1d = ps_c.tile([128, N], dt, tag="ctx1d")
            nc.tensor.matmul(ctx1d[:], (cs["W1cd"][:]), (ctxt[:]), start=True,
                             stop=True)
            score = ps_o.tile([128, 128], dt, tag="fin")
            for q in range(8):
                s1pair = []
                for p in (2 * q, 2 * q + 1):
                    on_act = (p % 16) < n_act_evict
                    t1 = s1p.tile([128, N], dtb, tag="s1a" if on_act else "s1v",
                                  name=f"s1_{n}_{p}", bufs=4)
                    if on_act:
                        sc.activation(t1[:], ctx1d[:], AF.Relu,
                                      bias=cs["abp"][:, p:p + 1])
                    else:
                        v.tensor_scalar(t1[:], ctx1d[:], cs["abp"][:, p:p + 1],
                                        0.0, OP.add, OP.max)
                    s1pair.append(t1)
                z2q = ps_z.tile([128, N], dt, tag="z2", name=f"z2_{n}_{q}")
                nc.tensor.matmul(z2q[0:64, :], (cs["W2blk"][:]), (s1pair[0][:]),
                                 start=True, stop=True)
                nc.tensor.matmul(z2q[64:128, :], (cs["W2blk"][:]),
                                 (s1pair[1][:]), start=True, stop=True)
                on_act2 = q < n_s2_act
                t = s2p.tile([128, N], dtb, tag="s2a" if on_act2 else "s2v",
                             name=f"s2_{n}_{q}", bufs=3)
                if on_act2:
                    sc.activation(t[:], z2q[:], AF.Relu, bias=cs["b2q"][:, 0:1])
                else:
                    v.tensor_scalar(t[:], z2q[:], cs["b2q"][:, 0:1], 0.0,
                                    OP.add, OP.max)
                for s in range(4):
                    nc.tensor.matmul(score[:, 32 * s + 4 * q:32 * s + 4 * q + 4],
                                     (t[:, 128 * s:128 * (s + 1)]),
                                     (cs["w3blk"][:]), start=True, stop=True)

            numer = ps_o.tile([128, 136], dt, tag="fin")
            for s in range(4):
                nc.tensor.matmul(numer[:, 34 * s:34 * (s + 1)],
                                 (expl[:, N * n + 128 * s:N * n + 128 * (s + 1)]),
                                 (cs["Bm1"][:]), start=True, stop=True)

            recipT = fout.tile([128, 4], dt, tag="recip")
            den = numer.rearrange("p (s c) -> p s c", c=34)[:, :, 32]
            v.reciprocal(recipT[:], den)
            tmp = fout.tile([128, 120], dt, tag="tmp")
            for s in range(4):
                v.tensor_scalar(tmp[:, 30 * s:30 * (s + 1)],
                                numer[:, 34 * s:34 * s + 30],
                                recipT[:, s:s + 1], None, OP.mult)
            outT = fout.tile([128, 120], dt, tag="outT")
            sc_ap = score.rearrange("p (s c) -> p s c", c=32)[:, :, 0:30]
            v.tensor_tensor(outT.rearrange("p (s c) -> p s c", c=30),
                            tmp.rearrange("p (s c) -> p s c", c=30),
                            sc_ap, OP.add)
            nc.sync.dma_start(out_r[n],
                              outT.rearrange("p (s c) -> p s c", c=30))

    nc.compile()
    return nc


def _get_program(consts):
    key = "prog"
    if key not in _cache:
        _cache[key] = _build(consts)
    return _cache[key]


def kernel(**inputs):
    in_maps, consts = _prep(inputs)
    nc = _get_program(consts)
    from concourse.bass_utils import run_bass_kernel_spmd
    res = run_bass_kernel_spmd(nc, in_maps, core_ids=list(range(NCORES)))
    out = np.concatenate([res.results[i]["out"] for i in range(NCORES)], 0)
    return out.astype(np.float32)


# revision 19
# speedup vs baseline: 2.2006x; 1.0607x over previous
"""Trainium2 Bass kernel for nn_EnhancedCardAwarePolicy.

Strategy: pure data-parallel across 8 NeuronCores (batch 16384 -> 2048/core).

Key algebraic simplifications (exactly value-preserving vs the reference):
  * The hand self-attention MHA is dead code: the cross-attention that
    consumes it has sequence length 1, so its softmax is identically 1 and
    its output is independent of the query.  hand_ctx reduces to
        (8 / max(hand_size,1)) * (enemy_emb @ he_wv @ he_wo + he_bv @ he_wo + he_bo)
  * Card encodings are pure functions of the card index 0..53 -> fold the
    embedding tables into one [54, 32] table, and fold that table through
    the downstream linear layers, so the enemy-card path becomes two
    matmuls against a one-hot [54, B] matrix built on-device.
  * strat_ctx's second linear layer is folded into cx_w1.
  * The per-action tables ([30,4] action_card_indices, replicated per the
    sharding hint) are folded into per-action bias vectors on the host.
  * softmax+bonus is computed unnormalized: out = score + (expl@Bm)/(expl@1),
    with the scalar bias as_b3 folded so one reciprocal+mul suffices.

Device layout: feature-major [D, B] activations; per-batch card features are
computed batch-major [128, 16] on GPSIMD and rotated into rows with PE
transposes.  The dominant cost (the [B, 30, 64/32] action MLP) runs as
2-actions-per-128-partition matmuls with relu+bias fused into the PSUM->SBUF
eviction, split across the Scalar and Vector engines.
"""

import numpy as np
from contextlib import ExitStack

B = 16384
NCORES = 8
BC = B // NCORES          # 2048 batch rows per core
NCH = 4                   # chunks per core
N = BC // NCH             # 512 batch columns per chunk
A = 30                    # real actions
AP_ = 32                  # padded actions
E = 32
HID = 128

_cache = {}


# ---------------------------------------------------------------------------
# host-side folding
# ---------------------------------------------------------------------------

def _card_table(val_emb, suit_emb, type_emb):
    """[54, 32] full card encoding table, matching _encode_cards."""
    c = np.arange(54)
    invalid = (c == 0) | (c == 53)
    v = np.where(invalid, 0, (c - 1) % 13 + 1)
    s = np.where(invalid, 0, (c - 1) // 13 + 1)
    ce = np.concatenate([val_emb[v], suit_emb[s]], axis=-1)          # [54, 32]
    ct = np.where(v == 11, 1, np.where(v == 12, 2, np.where(v == 13, 3, 0)))
    te = type_emb[ct]                                                # [54, 8]
    pad = np.zeros((54, E - te.shape[-1]), np.float32)
    return (ce + np.concatenate([te, pad], axis=-1)).astype(np.float32)


def _action_fold(ac, card_emb, ce_w1, ce_b1, ce_w2, ce_b2,
                 as_w1, as_b1, as_b3):
    """Per-action biases + bonus matrix from action_card_indices [30, 4]."""
    ac = np.asarray(ac, np.int64)
    mask = ac != 0
    combo_size = mask.sum(1).astype(np.float32)
    values = np.where(mask, (ac - 1) % 13 + 1, 0)
    has_valid = mask.any(1)
    fidx = np.argmax(mask, axis=1)
    fv = values[np.arange(ac.shape[0]), fidx]
    same = np.where(mask, values == fv[:, None], True).all(1).astype(np.float32)
    vf = values.astype(np.float32)
    attack = np.where(values == 1, 1.0,
             np.where(values == 11, 10.0,
             np.where(values == 12, 15.0,
             np.where(values == 13, 20.0, vf))))
    total = (attack * mask).sum(1).astype(np.float32)
    suits = np.where(mask, (ac - 1) // 13 + 1, 0)
    uniq = sum((suits == s).any(1) for s in (1, 2, 3, 4)).astype(np.float32)
    ace = ((values == 1) & mask).any(1).astype(np.float32)
    valid = ((combo_size <= 4.0) & ((same > 0) | (ace > 0))).astype(np.float32)
    feats = np.stack([combo_size, same, total, uniq, ace, valid], 1)
    feats = np.where(has_valid[:, None], feats, 0.0).astype(np.float32)

    emb = card_emb[ac]                                   # [30, 4, 32]
    m = mask.astype(np.float32)[..., None]
    cnt = np.maximum(m.sum(1), 1.0)
    act_emb = (emb * m).sum(1) / cnt
    act_emb = np.where(has_valid[:, None], act_emb, 0.0).astype(np.float32)
    combo_enc = np.maximum(feats @ ce_w1 + ce_b1, 0.0) @ ce_w2 + ce_b2

    action_bias = act_emb @ as_w1[HID:HID + E] + combo_enc @ as_w1[HID + E:] + as_b1

    strength = feats[:, 2] / 20.0
    b3 = float(as_b3[0])
    Bm1 = np.zeros((4, AP_ + 2), np.float32)
    for a in range(A):
        if has_valid[a]:
            col = np.array([strength[a], 1.0 - strength[a], 0.0, 0.0])
        else:
            col = np.array([0.0, 0.0, 0.0, 2.0])
        Bm1[:, a] = col + b3
    Bm1[:, AP_] = 1.0                                    # denominator column
    ab = np.zeros((AP_, 64), np.float32)
    ab[:A] = action_bias
    return ab, Bm1


def _prep(inputs):
    """Fold weights, build per-core input maps. Returns (in_maps, shapes)."""
    f32 = lambda x: np.ascontiguousarray(np.asarray(x), dtype=np.float32)
    hc = np.asarray(inputs["hand_cards"])        # [B, 8] int
    ec = np.asarray(inputs["enemy_card"])        # [B]
    hs = np.asarray(inputs["hand_size"])         # [B]
    gs = f32(inputs["game_state"])               # [B, 10]
    dc = f32(inputs["discard_pile_cards"])       # [B, 54]

    card_emb = _card_table(f32(inputs["val_emb"]), f32(inputs["suit_emb"]),
                           f32(inputs["type_emb"]))
    card_emb1 = np.concatenate([card_emb, np.ones((54, 1), np.float32)], 1)

    he_wv, he_bv = f32(inputs["he_wv"]), f32(inputs["he_bv"])
    he_wo, he_bo = f32(inputs["he_wo"]), f32(inputs["he_bo"])
    Mc = np.concatenate([he_wv @ he_wo, (he_bv @ he_wo + he_bo)[None]], 0)  # [33,32]
    A0s = 8.0 * (card_emb1 @ Mc)                                   # [54, 32]

    cx_w1, cx_b1 = f32(inputs["cx_w1"]), f32(inputs["cx_b1"])
    W1h = np.ascontiguousarray(cx_w1[0:E])                         # [32, 128]
    A2 = card_emb @ cx_w1[E:2 * E]                                 # [54, 128]
    W1s = cx_w1[2 * E:2 * E + 32]                                  # [32, 128]
    W1d = np.ascontiguousarray(cx_w1[2 * E + 32:])                 # [54, 128]
    se_w1, se_b1 = f32(inputs["se_w1"]).copy(), f32(inputs["se_b1"])
    se_w2, se_b2 = f32(inputs["se_w2"]), f32(inputs["se_b2"])
    U = se_w2 @ W1s                                                # [64, 128]
    b1f = cx_b1 + se_b2 @ W1s                                      # [128]
    se_w1[19] /= 4.0          # device computes suit-diversity count 0..4

    as_w1, as_b1 = f32(inputs["as_w1"]), f32(inputs["as_b1"])
    as_w2, as_b2 = f32(inputs["as_w2"]), f32(inputs["as_b2"])
    as_w3, as_b3 = f32(inputs["as_w3"]), f32(inputs["as_b3"])
    ab, Bm1 = _action_fold(inputs["action_card_indices"], card_emb,
                           f32(inputs["ce_w1"]), f32(inputs["ce_b1"]),
                           f32(inputs["ce_w2"]), f32(inputs["ce_b2"]),
                           as_w1, as_b1, as_b3)
    W1c = as_w1[:HID]                                              # [128, 64]
    W1cd = np.concatenate([W1c, W1c], 1)                           # [128, 128]
    abp = np.zeros((128, 16), np.float32)
    for p in range(16):
        abp[0:64, p] = ab[2 * p]
        abp[64:128, p] = ab[2 * p + 1]
    W2blk = np.zeros((128, 64), np.float32)
    W2blk[0:64, 0:32] = as_w2
    W2blk[64:128, 32:64] = as_w2
    b2q = np.tile(as_b2, 4)[:, None].astype(np.float32)            # [128, 1]
    w3blk = np.zeros((128, 4), np.float32)
    for i in range(4):
        w3blk[32 * i:32 * i + 32, i] = as_w3[:, 0]

    consts = {
        "iota54": np.arange(54, dtype=np.float32)[:, None],
        "ident": np.eye(128, dtype=np.float32),
        "A0s": A0s, "A2": np.ascontiguousarray(A2),
        "W1h": W1h, "U": np.ascontiguousarray(U),
        "W1d": W1d, "b1f": b1f[:, None],
        "sew1": np.concatenate([se_w1[0:10], np.zeros((22, 64), np.float32),
                                se_w1[10:20]], 0),
        "seb1": se_b1[:, None],
        "cxw2": f32(inputs["cx_w2"]), "cxb2": f32(inputs["cx_b2"])[:, None],
        "cxw3": f32(inputs["cx_w3"]), "cxb3": f32(inputs["cx_b3"])[:, None],
        "atw1": f32(inputs["atc_w1"]), "atb1": f32(inputs["atc_b1"])[:, None],
        "atw2": f32(inputs["atc_w2"]), "atb2": f32(inputs["atc_b2"])[:, None],
        "W1cd": W1cd, "abp": abp, "W2blk": W2blk, "b2q": b2q,
        "w3blk": w3blk, "Bm1": Bm1,
    }
    import ml_dtypes
    consts = {k: np.ascontiguousarray(v, dtype=np.float32) for k, v in consts.items()}
    for k in ("W2blk", "w3blk"):
        consts[k] = consts[k].astype(np.float16)

    in_maps = []
    for i in range(NCORES):
        sl = slice(i * BC, (i + 1) * BC)
        hci = hc[sl].astype(np.float32)
        m = dict(consts)
        m["ecT"] = np.ascontiguousarray(ec[sl].astype(np.float32)[None, :])
        gst = np.zeros((32, BC), np.float32)
        gst[0:10] = gs[sl].T
        m["gsT"] = gst
        m["dcT"] = np.ascontiguousarray(dc[sl].T)
        m["hcS"] = np.ascontiguousarray(
            hci.reshape(16, 128, 8).transpose(1, 0, 2).reshape(128, 128))
        m["hsBM"] = np.ascontiguousarray(
            hs[sl].astype(np.float32).reshape(16, 128).T)
        in_maps.append(m)
    return in_maps, consts


# ---------------------------------------------------------------------------
# device program
# ---------------------------------------------------------------------------

def _build(consts, n_act_evict=11, n_s2_act=6):
    import concourse.bass as bass
    import concourse.tile as tile
    import concourse.mybir as mybir
    from concourse import bacc

    dt = mybir.dt.float32
    dtb = mybir.dt.float16
    dtr = mybir.dt.float32r
    rr = lambda ap: ap.bitcast(dtr)
    AF = mybir.ActivationFunctionType
    OP = mybir.AluOpType
    AX = mybir.AxisListType

    nc = bacc.Bacc("TRN2", target_bir_lowering=False, debug=False,
                   enable_asserts=False, num_devices=NCORES)

    MM_CONSTS = {"sew1", "A0s", "A2", "W1h", "U", "W1d", "cxw2", "cxw3",
                 "atw1", "atw2", "W1cd", "Bm1"}
    BF_CONSTS = {"W2blk", "w3blk"}
    def cdt(name):
        return dtr if name in MM_CONSTS else (dtb if name in BF_CONSTS else dt)
    din = {}
    for name, arr in consts.items():
        din[name] = nc.dram_tensor(name, list(arr.shape), cdt(name),
                                   kind="ExternalInput").ap()
    for name, shape in (("ecT", [1, BC]), ("gsT", [32, BC]), ("dcT", [54, BC]),
                        ("hcS", [128, 128]), ("hsBM", [128, 16])):
        din[name] = nc.dram_tensor(name, shape,
                                   dtr if name in ("gsT", "dcT") else dt,
                                   kind="ExternalInput").ap()
    out_d = nc.dram_tensor("out", [BC, A], dt, kind="ExternalOutput").ap()
    # out rows b = 512*n + 128*s + p  ->  [n][p, s, a]
    out_r = out_d.rearrange("(n s p) a -> n p s a", n=NCH, s=4, p=128)

    with tile.TileContext(nc) as tc, ExitStack() as ctx:
        cpool = ctx.enter_context(tc.tile_pool(name="consts", bufs=1))
        core = ctx.enter_context(tc.tile_pool(name="core", bufs=1))
        work = ctx.enter_context(tc.tile_pool(name="work", bufs=3))
        s1p = ctx.enter_context(tc.tile_pool(name="s1p", bufs=8))
        s2p = ctx.enter_context(tc.tile_pool(name="s2p", bufs=4))
        fout = ctx.enter_context(tc.tile_pool(name="fout", bufs=2))
        ps_f = ctx.enter_context(tc.tile_pool(name="ps_f", bufs=2, space="PSUM"))
        ps_c = ctx.enter_context(tc.tile_pool(name="ps_c", bufs=2, space="PSUM"))
        ps_z = ctx.enter_context(tc.tile_pool(name="ps_z", bufs=2, space="PSUM"))
        ps_o = ctx.enter_context(tc.tile_pool(name="ps_o", bufs=2, space="PSUM"))

        # ---- constants into SBUF
        cs = {}
        for k2, (name, arr) in enumerate(consts.items()):
            t = cpool.tile(list(arr.shape), cdt(name), tag=name, name="c_" + name)
            eng = nc.sync if k2 % 2 == 0 else nc.gpsimd
            eng.dma_start(t[:], din[name])
            cs[name] = t

        # ---- core-level tiles
        sh_in = core.tile([44, BC], dtr, tag="sh_in")    # strat_in rows + r row
        nc.gpsimd.dma_start(sh_in[0:32, :], din["gsT"])
        dcT = core.tile([54, BC], dtr, tag="dcT")
        nc.sync.dma_start(dcT[:], din["dcT"])

        hcS = core.tile([128, 128], dt, tag="hcS")
        nc.gpsimd.dma_start(hcS[:], din["hcS"])
        S = core.tile([128, 176], dt, tag="S")          # 11 blocks of 16
        nc.sync.dma_start(S[:, 0:16], din["hsBM"])
        expl = core.tile([4, BC], dtr, tag="expl")
        ecbc = core.tile([54, BC], dt, tag="ecbc")
        nc.sync.dma_start(ecbc[:], bass.AP(din["ecT"].tensor, din["ecT"].offset,
                                           [[0, 54], [1, BC]]))

        # ---- per-card features (GPSIMD, batch-major, exact integer ops)
        ft = {k: core.tile([128, 128], dt, tag=f"ft_{k}", name=f"ft_{k}") for k in
              ("t", "g13", "g26", "g39", "s0", "m13", "v0", "mask",
               "s0p", "ace", "face", "lowd", "low", "su1", "su2", "su3", "su4")}
        g = nc.vector
        g.tensor_scalar(ft["t"][:], hcS[:], -1.0, None, OP.add)
        g.tensor_scalar(ft["g13"][:], ft["t"][:], 13.0, None, OP.is_ge)
        g.tensor_scalar(ft["g26"][:], ft["t"][:], 26.0, None, OP.is_ge)
        g.tensor_scalar(ft["g39"][:], ft["t"][:], 39.0, None, OP.is_ge)
        g.tensor_tensor(ft["s0"][:], ft["g13"][:], ft["g26"][:], OP.add)
        g.tensor_tensor(ft["s0"][:], ft["s0"][:], ft["g39"][:], OP.add)
        g.tensor_scalar(ft["m13"][:], ft["s0"][:], 13.0, None, OP.mult)
        g.tensor_tensor(ft["v0"][:], ft["t"][:], ft["m13"][:], OP.subtract)
        g.tensor_scalar(ft["mask"][:], hcS[:], 0.5, None, OP.is_ge)
        g.tensor_scalar(ft["s0p"][:], ft["s0"][:], 1.0, None, OP.add)
        g.tensor_tensor(ft["s0p"][:], ft["s0p"][:], ft["mask"][:], OP.mult)
        g.tensor_scalar(ft["ace"][:], ft["v0"][:], 0.0, None, OP.is_equal)
        g.tensor_scalar(ft["face"][:], ft["v0"][:], 10.0, None, OP.is_ge)
        g.tensor_scalar(ft["lowd"][:], ft["v0"][:], 1.0, None, OP.is_ge)
        g.tensor_scalar(ft["low"][:], ft["v0"][:], 5.0, None, OP.is_le)
        g.tensor_tensor(ft["low"][:], ft["low"][:], ft["lowd"][:], OP.mult)
        for k, s in (("su1", 1.0), ("su2", 2.0), ("su3", 3.0), ("su4", 4.0)):
            g.tensor_scalar(ft[k][:], ft["s0p"][:], s, None, OP.is_equal)

        # ---- reduce 8 cards -> per-batch sums into S blocks (DVE)
        v = nc.vector
        for blk, k in ((1, "ace"), (2, "face"), (3, "low"),
                       (4, "su1"), (5, "su2"), (6, "su3"), (7, "su4")):
            src = ft[k].rearrange("p (j c) -> p j c", c=8)
            v.tensor_reduce(S[:, 16 * blk:16 * blk + 16], src, AX.X, OP.add)

        # ---- hvr, sdiv, r (batch-major small tiles)
        hsr = core.tile([128, 16], dt, tag="hsr")
        v.tensor_scalar(hsr[:], S[:, 0:16], 1e-8, None, OP.add)
        v.reciprocal(hsr[:], hsr[:])
        v.tensor_tensor(S[:, 128:144], S[:, 32:48], hsr[:], OP.mult)  # hvr
        ge = [core.tile([128, 16], dt, tag=f"ge{k}", name=f"ge{k}") for k in range(4)]
        for k in range(4):
            v.tensor_scalar(ge[k][:], S[:, 64 + 16 * k:80 + 16 * k], 0.5, None,
                            OP.is_ge)
        v.tensor_tensor(ge[0][:], ge[0][:], ge[1][:], OP.add)
        v.tensor_tensor(ge[2][:], ge[2][:], ge[3][:], OP.add)
        v.tensor_tensor(S[:, 144:160], ge[0][:], ge[2][:], OP.add)   # sdiv cnt
        rmax = core.tile([128, 16], dt, tag="rmax")
        v.tensor_scalar(rmax[:], S[:, 0:16], 1.0, None, OP.max)
        v.reciprocal(S[:, 160:176], rmax[:])                          # r

        sc = nc.scalar
        iota_col = cs["iota54"][:, 0:1]

        for n in range(NCH):
            cols = slice(N * n, N * (n + 1))

            # -- rotate per-batch scalars into rows: 4 transposes of [128, 11]
            scalT = ps_f.tile([128, N], dt, tag="fe", name="scalT")
            S_kj = S.rearrange("p (k j) -> p j k", j=16)
            for s in range(4):
                nc.tensor.transpose(scalT[0:11, 128 * s:128 * (s + 1)],
                                    S_kj[:, 4 * n + s, :], cs["ident"][:])
            sc.activation(sh_in[32:43, cols], scalT[0:11, :], AF.Copy)

            # -- strat hidden layer
            shp = ps_f.tile([128, N], dt, tag="fe")
            nc.tensor.matmul(shp[0:64, :], (cs["sew1"][:]), (sh_in[0:42, cols]),
                             start=True, stop=True)
            sh = work.tile([64, N], dtr, tag="sh")
            sc.activation(sh[:], shp[0:64, :], AF.Relu, bias=cs["seb1"][:, 0:1])

            # -- enemy one-hot
            oh = work.tile([54, N], dtr, tag="oh")
            v.tensor_scalar(oh[:], ecbc[:, cols], iota_col, None, OP.is_equal)

            # -- hand_ctx = (A0s^T oh) * r
            rrow = work.tile([1, N], dtr, tag="rrow")
            nc.sync.dma_start(rrow[:], sh_in[42:43, cols])
            r32 = work.tile([32, N], dtr, tag="r32")
            nc.gpsimd.partition_broadcast(r32[:], rrow[:], channels=32)
            yps = ps_f.tile([128, N], dt, tag="fe")
            nc.tensor.matmul(yps[0:32, :], (cs["A0s"][:]), (oh[:]),
                             start=True, stop=True)
            hctx = work.tile([32, N], dtr, tag="hctx")
            v.tensor_tensor(hctx[:], yps[0:32, :], r32[:], OP.mult)

            # -- z1 = A2^T oh + W1h^T hctx + U^T sh + W1d^T dcT + b1f
            z1 = ps_f.tile([128, N], dt, tag="fe")
            nc.tensor.matmul(z1[:], (cs["A2"][:]), (oh[:]), start=True, stop=False)
            nc.tensor.matmul(z1[:], (cs["W1h"][:]), (hctx[:]), start=False, stop=False)
            nc.tensor.matmul(z1[:], (cs["U"][:]), (sh[:]), start=False, stop=False)
            nc.tensor.matmul(z1[:], (cs["W1d"][:]), (dcT[:, cols]), start=False,
                             stop=True)
            h1 = work.tile([128, N], dtr, tag="h1")
            sc.activation(h1[:], z1[:], AF.Relu, bias=cs["b1f"][:, 0:1])

            h2p = ps_f.tile([128, N], dt, tag="fe")
            nc.tensor.matmul(h2p[:], (cs["cxw2"][:]), (h1[:]), start=True, stop=True)
            h2 = work.tile([128, N], dtr, tag="h2")
            sc.activation(h2[:], h2p[:], AF.Relu, bias=cs["cxb2"][:, 0:1])

            ctxp = ps_f.tile([128, N], dt, tag="fe")
            nc.tensor.matmul(ctxp[:], (cs["cxw3"][:]), (h2[:]), start=True, stop=True)
            ctxt = work.tile([128, N], dtr, tag="ctxt")
            sc.activation(ctxt[:], ctxp[:], AF.Identity, bias=cs["cxb3"][:, 0:1])

            # -- action-type probs (unnormalized exp)
            tphp = ps_f.tile([128, N], dt, tag="fe")
            nc.tensor.matmul(tphp[0:64, :], (cs["atw1"][:]), (ctxt[:]),
                             start=True, stop=True)
            tph = work.tile([64, N], dtr, tag="tph")
            sc.activation(tph[:], tphp[0:64, :], AF.Relu, bias=cs["atb1"][:, 0:1])
            tlp = ps_f.tile([128, N], dt, tag="fe")
            nc.tensor.matmul(tlp[0:4, :], (cs["atw2"][:]), (tph[:]),
                             start=True, stop=True)
            sc.activation(expl[:, cols], tlp[0:4, :], AF.Exp,
                          bias=cs["atb2"][:, 0:1])

            # -- action MLP
            ctx1d = ps_c.tile([128, N], dt, tag="ctx1d")
            nc.tensor.matmul(ctx1d[:], (cs["W1cd"][:]), (ctxt[:]), start=True,
                             stop=True)
            score = ps_o.tile([128, 128], dt, tag="fin")
            for q in range(8):
                s1pair = []
                for p in (2 * q, 2 * q + 1):
                    on_act = (p % 16) < n_act_evict
                    t1 = s1p.tile([128, N], dtb, tag="s1a" if on_act else "s1v",
                                  name=f"s1_{n}_{p}", bufs=4)
                    if on_act:
                        sc.activation(t1[:], ctx1d[:], AF.Relu,
                                      bias=cs["abp"][:, p:p + 1])
                    else:
                        v.tensor_scalar(t1[:], ctx1d[:], cs["abp"][:, p:p + 1],
                                        0.0, OP.add, OP.max)
                    s1pair.append(t1)
                z2q = ps_z.tile([128, N], dt, tag="z2", name=f"z2_{n}_{q}")
                nc.tensor.matmul(z2q[0:64, :], (cs["W2blk"][:]), (s1pair[0][:]),
                                 start=True, stop=True)
                nc.tensor.matmul(z2q[64:128, :], (cs["W2blk"][:]),
                                 (s1pair[1][:]), start=True, stop=True)
                on_act2 = q < n_s2_act
                t = s2p.tile([128, N], dtb, tag="s2a" if on_act2 else "s2v",
                             name=f"s2_{n}_{q}", bufs=3)
                if on_act2:
                    sc.activation(t[:], z2q[:], AF.Relu, bias=cs["b2q"][:, 0:1])
                else:
                    v.tensor_scalar(t[:], z2q[:], cs["b2q"][:, 0:1], 0.0,
                                    OP.add, OP.max)
                for s in range(4):
                    nc.tensor.matmul(score[:, 32 * s + 4 * q:32 * s + 4 * q + 4],
                                     (t[:, 128 * s:128 * (s + 1)]),
                                     (cs["w3blk"][:]), start=True, stop=True)

            numer = ps_o.tile([128, 136], dt, tag="fin")
            for s in range(4):
                nc.tensor.matmul(numer[:, 34 * s:34 * (s + 1)],
                                 (expl[:, N * n + 128 * s:N * n + 128 * (s + 1)]),
                                 (cs["Bm1"][:]), start=True, stop=True)

            recipT = fout.tile([128, 4], dt, tag="recip")
            den = numer.rearrange("p (s c) -> p s c", c=34)[:, :, 32]
            v.reciprocal(recipT[:], den)
            tmp = fout.tile([128, 120], dt, tag="tmp")
            for s in range(4):
                v.tensor_scalar(tmp[:, 30 * s:30 * (s + 1)],
                                numer[:, 34 * s:34 * s + 30],
                                recipT[:, s:s + 1], None, OP.mult)
            outT = fout.tile([128, 120], dt, tag="outT")
            sc_ap = score.rearrange("p (s c) -> p s c", c=32)[:, :, 0:30]
            v.tensor_tensor(outT.rearrange("p (s c) -> p s c", c=30),
                            tmp.rearrange("p (s c) -> p s c", c=30),
                            sc_ap, OP.add)
            (nc.sync if n % 2 == 0 else nc.gpsimd).dma_start(
                out_r[n], outT.rearrange("p (s c) -> p s c", c=30))

    nc.compile()
    return nc


def _get_program(consts):
    key = "prog"
    if key not in _cache:
        _cache[key] = _build(consts)
    return _cache[key]


def kernel(**inputs):
    in_maps, consts = _prep(inputs)
    nc = _get_program(consts)
    from concourse.bass_utils import run_bass_kernel_spmd
    res = run_bass_kernel_spmd(nc, in_maps, core_ids=list(range(NCORES)))
    out = np.concatenate([res.results[i]["out"] for i in range(NCORES)], 0)
    return out.astype(np.float32)
